# revision 22
# baseline (speedup 1.0000x reference)
"""Trainium2 Bass kernel for nn_ConvTranBackbone (conv tokenizer + 4-layer
transformer encoder). Data-parallel over batch: 16 batch elems -> 8 cores x 2.

V2 design (vs v1 baseline):
- bf16 weights + activations for all transformer matmuls (fast LDWEIGHTS/FWL);
  residual stream stays fp32. Conv1 fp32, conv2 bf16.
- Relative-position bias applied multiplicatively after exp (probs =
  exp(s) * exp(bias)) via a shifted bf16 strip multiply on DVE, or fused
  into a one-op DVE "Schraudolph" exp (scores*A + strip -> int16, bitcast
  bf16) for SCHRAUD_HEADS. No identity-matmul bias strips on the PE.
- LN rstd via ACT Ln->Exp (one activation table set shared with attention
  exp); DVE reciprocal replaced by reciprocal_approx_fast for softmax Z.
- Per-b stage pipelining: the two batch elements' stages are emitted
  interleaved so PE matmuls overlap the other stream's elementwise chains
  (keeps the PE HAM clock-gate warm).
- PSUM: 'mm' [128,2,512]x2 + 'sc' [128,512]x2 + 'cz' [128,2,512]x1 = 8 banks.
"""
import sys
import math

sys.path.insert(0, '/opt/trn_rl_repo')

import numpy as np
import ml_dtypes

import concourse.bass as bass
import concourse.bacc as bacc
import concourse.mybir as mybir
import concourse.tile as tile
from concourse.bass_utils import run_bass_kernel_spmd

F32 = mybir.dt.float32
F32R = mybir.dt.float32r
BF16 = mybir.dt.bfloat16
I16 = mybir.dt.int16
AF = mybir.ActivationFunctionType
ALU = mybir.AluOpType

B, C_IN, S, D, H, L, FF = 16, 32, 512, 256, 8, 4, 1024
HD = D // H          # 32
EPS = 1e-5
NCORES = 8
BLOC = B // NCORES   # 2 batch elems per core
DB = 2               # d blocks of 128
TOK = BLOC * S       # 1024 tokens per core

# Schraudolph exp constants (bf16 flavor: y = round(x*SCA + SCB) as int16,
# bitcast to bf16). The constant offset cancels in softmax normalization.
SCA = float((1 << 7) / math.log(2.0))
SCB = float(127 * 128 - 0.043 * 128)
SCHRAUD_HEADS = (3, 7)          # global head idx -> probs via DVE fused exp
ACT_HEADS = tuple(h for h in range(H) if h not in SCHRAUD_HEADS)
E_IDX = {h: i for i, h in enumerate(ACT_HEADS)}
S_IDX = {h: i for i, h in enumerate(SCHRAUD_HEADS)}
NE, NS = len(ACT_HEADS), len(SCHRAUD_HEADS)

# engine knobs: fraction of exp-strip multiplies routed to gpsimd, by jc
MULT_GPS_JC = ()     # e.g. (0, 2) -> jc 0,2 multiplies on gpsimd

TRACE = False
_CACHE = {}


# ---------------------------------------------------------------- host prep
def _pos_encoding():
    pos = np.arange(S, dtype=np.float32)[:, None]
    div = np.exp(np.arange(0, D, 2, dtype=np.float32) * (-math.log(10000.0) / D))
    scale = D / S
    pe = np.zeros((S, D), dtype=np.float32)
    pe[:, 0::2] = np.sin(pos * div * scale)
    pe[:, 1::2] = np.cos(pos * div * scale)
    return pe


def _prep(inp):
    f = lambda x: np.ascontiguousarray(np.asarray(x, np.float32))
    bf = lambda x: np.ascontiguousarray(np.asarray(x, ml_dtypes.bfloat16))
    p = {}
    s1 = f(inp['bn1_g']) / np.sqrt(np.float32(1.0) + np.float32(EPS))
    b1c = f(inp['conv1_b']) * s1 + f(inp['bn1_b'])
    s2 = f(inp['bn2_g']) / np.sqrt(np.float32(1.0) + np.float32(EPS))
    b2c = f(inp['conv2_b']) * s2 + f(inp['bn2_b'])
    cvec = np.zeros((128, DB, 4), np.float32)
    for db in range(DB):
        cvec[:, db, 0] = s1[db * 128:(db + 1) * 128]
        cvec[:, db, 1] = b1c[db * 128:(db + 1) * 128]
        cvec[:, db, 2] = s2[db * 128:(db + 1) * 128]
        cvec[:, db, 3] = b2c[db * 128:(db + 1) * 128]
    p['cvec'] = cvec

    w1 = f(inp['conv1_w'])
    w1A = np.zeros((128, D), np.float32)
    for kk in range(4):
        w1A[32 * kk:32 * kk + 32, :] = w1[:, :, kk].T
    w1B = np.zeros((96, D), np.float32)
    for j in range(3):
        w1B[32 * j:32 * j + 32, :] = w1[:, :, 4 + j].T
    p['w1A'], p['w1B'] = w1A, np.ascontiguousarray(w1B)

    w2 = f(inp['conv2_w'])
    w2t = np.zeros((128, DB, 5, D), np.float32)
    for cb in range(DB):
        for k in range(5):
            w2t[:, cb, k, :] = w2[:, cb * 128:(cb + 1) * 128, k].T
    p['w2t'] = w2t.astype(ml_dtypes.bfloat16)

    pe = _pos_encoding()
    p['peT'] = np.ascontiguousarray(pe.T.reshape(DB, 128, S).transpose(1, 0, 2))

    sc = np.float32(HD ** -0.5)
    for l in range(L):
        g1, b1l = f(inp['ln1_g'][l]), f(inp['ln1_b'][l])
        g2, b2l = f(inp['ln2_g'][l]), f(inp['ln2_b'][l])
        wq = f(inp['wq'][l]) * sc
        wk, wv, wo = f(inp['wk'][l]), f(inp['wv'][l]), f(inp['wo'][l])
        wm = np.zeros((128, 3, DB, D), np.float32)
        for i, w in enumerate([g1[:, None] * wq, g1[:, None] * wk,
                               g1[:, None] * wv]):
            for kb in range(DB):
                wm[:, i, kb, :] = w[kb * 128:(kb + 1) * 128, :]
        p[f'wqkvo{l}'] = wm.astype(ml_dtypes.bfloat16)
        # out-projection weights permuted to read normalized ctx bank tiles
        # directly: ctxn[p, pb] rows 0:32 hold head 4*hb+2*pb, rows 64:96
        # head 4*hb+2*pb+1, rows 32:64/96:128 are Z junk (zero weight).
        woP = np.zeros((128, 2, 2, D), np.float32)
        for hb in range(2):
            for pb in range(2):
                d0 = 128 * hb + 64 * pb
                woP[0:32, hb, pb, :] = wo[d0:d0 + 32, :]
                woP[64:96, hb, pb, :] = wo[d0 + 32:d0 + 64, :]
        p[f'woP{l}'] = woP.astype(ml_dtypes.bfloat16)
        p[f'bvb{l}'] = np.tile((b1l @ wv)[None, :], (128, 1)).astype(
            ml_dtypes.bfloat16)
        w1f = f(inp['w1'][l])
        w1m = np.zeros((128, DB, FF), np.float32)
        w1e = g2[:, None] * w1f
        for kb in range(DB):
            w1m[:, kb, :] = w1e[kb * 128:(kb + 1) * 128, :]
        p[f'wff1{l}'] = w1m.astype(ml_dtypes.bfloat16)
        w2f = f(inp['w2'][l])
        w2m = np.zeros((128, 8, D), np.float32)
        for kb in range(8):
            w2m[:, kb, :] = w2f[kb * 128:(kb + 1) * 128, :]
        p[f'wff2{l}'] = w2m.astype(ml_dtypes.bfloat16)
        # per-partition bias pack: cols [bq(2), bk(2), bo(2), b2(2), b1(8)]
        pv = np.zeros((128, 16), np.float32)
        bq, bk = b1l @ wq, b1l @ wk
        bo, b2v = f(inp['bo'][l]), f(inp['b2'][l])
        b1e = b2l @ w1f + f(inp['b1'][l])
        for db in range(DB):
            pv[:, 0 + db] = bq[db * 128:(db + 1) * 128]
            pv[:, 2 + db] = bk[db * 128:(db + 1) * 128]
            pv[:, 4 + db] = bo[db * 128:(db + 1) * 128]
            pv[:, 6 + db] = b2v[db * 128:(db + 1) * 128]
        for fb in range(8):
            pv[:, 8 + fb] = b1e[fb * 128:(fb + 1) * 128]
        p[f'pvec{l}'] = pv
        # shifted strips: strip[p, h, c] corresponds to tab[c - p, h]
        tab = f(inp['bias_table'][l])            # [2S-1, H]
        est = np.ones((128, NE, 1024), np.float32)
        sst = np.full((128, NS, 1024), SCB, np.float32)
        for pp in range(128):
            hi = min(1024, pp + 2 * S - 1)
            for h in ACT_HEADS:
                est[pp, E_IDX[h], pp:hi] = np.exp(tab[0:hi - pp, h])
            for h in SCHRAUD_HEADS:
                sst[pp, S_IDX[h], pp:hi] = tab[0:hi - pp, h] * SCA + SCB
        p[f'estrip{l}'] = est.astype(ml_dtypes.bfloat16)
        p[f'sstrip{l}'] = sst
    fvec = np.zeros((128, 4), np.float32)
    for db in range(DB):
        fvec[:, 0 + db] = f(inp['fn_g'])[db * 128:(db + 1) * 128]
        fvec[:, 2 + db] = f(inp['fn_b'])[db * 128:(db + 1) * 128]
    p['fvec'] = fvec
    p['identb'] = np.eye(128, dtype=ml_dtypes.bfloat16)
    p['identf'] = np.eye(128, dtype=np.float32)
    p['onesd'] = np.full((128, 128), 1.0 / 256.0, np.float32)
    p['zeros16'] = np.zeros((128, 16), np.float32)
    # vz slot template: per head slot h, Z-ones at cols [64*(h%2)+32, +32)
    vzt = np.zeros((128, 4, 4, 128), np.float32)
    for h in range(4):
        par = h % 2
        vzt[:, :, h, 64 * par + 32:64 * par + 64] = 1.0
    p['vztmpl'] = vzt.astype(ml_dtypes.bfloat16)
    return p


# ---------------------------------------------------------------- device build
def _pin_act_tables(nc):
    """Steer the act-table-load pass to natural_log_exp_and_others for
    Exp/Ln/Square (it picks the first set containing each function, which
    otherwise thrashes exp_and_others <-> natural_log on every LayerNorm).
    Only set *contents* are edited, never list order, so act_func_set_id
    indices stay aligned with act_info.json."""
    from concourse.hw_specs import get_activation_tables
    tabs = get_activation_tables(nc.m.arch)
    keep = {AF.Exp, AF.Ln, AF.Square}
    for name in list(tabs):
        if name == 'natural_log_exp_and_others':
            break
        tabs[name] -= keep


def _build(repeat=1, upto='full'):
    nc = bacc.Bacc()
    _pin_act_tables(nc)
    din = {}

    def dinp(name, shape, dt=F32R):
        din[name] = nc.dram_tensor(name, list(shape), dt, kind='ExternalInput')
        return din[name]

    x = dinp('x', [BLOC, C_IN, S])
    w1A = dinp('w1A', [128, D])
    w1B = dinp('w1B', [96, D])
    w2t = dinp('w2t', [128, DB, 5, D], BF16)
    cvec = dinp('cvec', [128, DB, 4], F32)
    peT = dinp('peT', [128, DB, S], F32)
    identb = dinp('identb', [128, 128], BF16)
    identf = dinp('identf', [128, 128], F32)
    onesd = dinp('onesd', [128, 128], F32R)
    zeros16 = dinp('zeros16', [128, 16], F32R)
    vztmpl = dinp('vztmpl', [128, 4, 4, 128], BF16)
    fvec = dinp('fvec', [128, 4], F32)
    for l in range(L):
        dinp(f'wqkvo{l}', [128, 3, DB, D], BF16)
        dinp(f'woP{l}', [128, 2, 2, D], BF16)
        dinp(f'wff1{l}', [128, DB, FF], BF16)
        dinp(f'wff2{l}', [128, 8, D], BF16)
        dinp(f'bvb{l}', [128, D], BF16)
        dinp(f'pvec{l}', [128, 16], F32)
        dinp(f'estrip{l}', [128, NE, 1024], BF16)
        dinp(f'sstrip{l}', [128, NS, 1024], F32)
    out = nc.dram_tensor('out', [BLOC, S, D], F32, kind='ExternalOutput')

    tc_cm = tile.TileContext(nc)
    tc = tc_cm.__enter__()
    cst = tc.alloc_tile_pool(name='cst', bufs=1)
    wp = tc.alloc_tile_pool(name='wp', bufs=2)
    ap = tc.alloc_tile_pool(name='ap', bufs=1)
    tp = tc.alloc_tile_pool(name='tp', bufs=2)
    vzp = tc.alloc_tile_pool(name='vzp', bufs=2)
    prp = tc.alloc_tile_pool(name='prp', bufs=6)
    h1p = tc.alloc_tile_pool(name='h1p', bufs=12)
    xip = tc.alloc_tile_pool(name='xip', bufs=4)
    zp = tc.alloc_tile_pool(name='zp', bufs=2)
    cnp = tc.alloc_tile_pool(name='cnp', bufs=4)
    ps = tc.alloc_tile_pool(name='ps', bufs=2, space='PSUM')

    # ---- consts
    identb_s = cst.tile([128, 128], BF16)
    nc.sync.dma_start(identb_s[:], identb[:])
    identf_s = cst.tile([128, 128], F32)
    nc.sync.dma_start(identf_s[:], identf[:])
    onesd_s = cst.tile([128, 128], F32R)
    nc.sync.dma_start(onesd_s[:], onesd[:])
    z16_s = cst.tile([128, 16], F32R)
    nc.sync.dma_start(z16_s[:], zeros16[:])
    cvec_s = cst.tile([128, DB, 4], F32)
    nc.sync.dma_start(cvec_s[:], cvec[:])
    fvec_s = cst.tile([128, 4], F32)
    nc.sync.dma_start(fvec_s[:], fvec[:])
    peT_s = cst.tile([128, DB, S], F32)
    nc.sync.dma_start(peT_s[:], peT[:])
    eps_s = cst.tile([128, 1], F32)
    nc.vector.memset(eps_s[:], EPS)
    w1A_s = cst.tile([128, D], F32R)
    nc.sync.dma_start(w1A_s[:], w1A[:])
    w1B_s = cst.tile([96, D], F32R)
    nc.sync.dma_start(w1B_s[:], w1B[:])
    w2t_s = cst.tile([128, DB, 5, D], BF16)
    nc.sync.dma_start(w2t_s[:], w2t[:])

    # vz slot templates (zeros + Z-ones). v columns are rewritten per use;
    # the static template regions persist across pool-slot reuse.
    for i in range(2):
        vzt_t = vzp.tile([128, 4, 4, 128], BF16, tag='vz', name=f'vzi{i}')
        nc.sync.dma_start(vzt_t[:], vztmpl[:])
    # zrec slots: Z-reciprocal rows are stream-shuffled in per use; the junk
    # rows (32:64, 96:128) are memset once so the out-of-band lanes of the
    # normalize multiply stay finite (their woP rows are zero).
    for i in range(2):
        zr_t = zp.tile([128, 2, 512], F32, tag='zrec', name=f'zri{i}')
        nc.vector.memset(zr_t[32:64, :, :], 1.0)
        nc.vector.memset(zr_t[96:128, :, :], 1.0)

    # persistent residual stream, feature-major [d mod 128, d//128, token]
    rt = ap.tile([128, DB, TOK], F32R)

    def emit_body(R):
        # ---------------- conv tokenizer (conv1 fp32, conv2 bf16)
        xts = {}
        for b in range(BLOC):
            X4 = xip.tile([128, 512], F32R, tag='xi', name=f'{R}x4_{b}')
            nc.sync.dma_start(X4[0:32, 3:512], x[b, :, 0:509])
            nc.sync.dma_start(X4[32:64, 2:512], x[b, :, 0:510])
            nc.sync.dma_start(X4[64:96, 1:512], x[b, :, 0:511])
            nc.sync.dma_start(X4[96:128, 0:512], x[b, :, 0:512])
            nc.sync.dma_start(X4[0:32, 0:3], z16_s[0:32, 0:3])
            nc.sync.dma_start(X4[32:64, 0:2], z16_s[32:64, 0:2])
            nc.sync.dma_start(X4[64:96, 0:1], z16_s[64:96, 0:1])
            X3 = xip.tile([128, 512], F32R, tag='xi', name=f'{R}x3_{b}')
            nc.sync.dma_start(X3[0:32, 0:511], x[b, :, 1:512])
            nc.sync.dma_start(X3[32:64, 0:510], x[b, :, 2:512])
            nc.sync.dma_start(X3[64:96, 0:509], x[b, :, 3:512])
            nc.sync.dma_start(X3[0:32, 511:512], z16_s[0:32, 0:1])
            nc.sync.dma_start(X3[32:64, 510:512], z16_s[32:64, 0:2])
            nc.sync.dma_start(X3[64:96, 509:512], z16_s[64:96, 0:3])
            xts[b] = (X4, X3)
        for b in range(BLOC):
            X4, X3 = xts[b]
            hp = tp.tile([128, DB, 516], BF16, tag='hp', name=f'{R}hp_{b}')
            for dc in range(DB):
                c1 = ps.tile([128, 512], F32, tag='mm', name=f'{R}c1_{b}{dc}')
                nc.tensor.matmul(c1[:], w1A_s[:, dc * 128:(dc + 1) * 128],
                                 X4[:], start=True, stop=False,
                                 skip_group_check=True)
                nc.tensor.matmul(c1[:], w1B_s[:, dc * 128:(dc + 1) * 128],
                                 X3[0:96, :], start=False, stop=True,
                                 skip_group_check=True)
                nc.gpsimd.memset(hp[:, dc, 0:2], 0.0)
                nc.gpsimd.memset(hp[:, dc, 514:516], 0.0)
                nc.scalar.activation(hp[:, dc, 2:514], c1[:], AF.Gelu,
                                     bias=cvec_s[:, dc, 1:2],
                                     scale=cvec_s[:, dc, 0:1])
            for dc in range(DB):
                c2 = ps.tile([128, 512], F32, tag='mm', name=f'{R}c2_{b}{dc}')
                for cb in range(DB):
                    for k in range(5):
                        nc.tensor.matmul(
                            c2[:], w2t_s[:, cb, k, dc * 128:(dc + 1) * 128],
                            hp[:, cb, k:k + 512],
                            start=(cb == 0 and k == 0),
                            stop=(cb == 1 and k == 4), skip_group_check=True)
                tg = h1p.tile([128, 512], BF16, tag='h1', name=f'{R}tg_{b}_{dc}')
                nc.scalar.activation(tg[:], c2[:], AF.Gelu,
                                     bias=cvec_s[:, dc, 3:4],
                                     scale=cvec_s[:, dc, 2:3])
                nc.vector.tensor_add(rt[:, dc, b * S:(b + 1) * S],
                                     tg[:], peT_s[:, dc, :])

        # ---------------- per-(layer, b) stage emitters
        def layernorm(b, xn_t, tag, fin_stats=None):
            sl = slice(b * S, (b + 1) * S)
            sq = tp.tile([128, DB, 512], F32R, tag='sq', name=f'{R}sq_{tag}', bufs=1)
            nc.gpsimd.tensor_tensor(sq[:], rt[:, :, sl], rt[:, :, sl], ALU.mult)
            mu = ps.tile([128, 512], F32, tag='mm', name=f'{R}mu_{tag}')
            for db in range(DB):
                nc.tensor.matmul(mu[:], onesd_s[:], rt[:, db, sl],
                                 start=(db == 0), stop=(db == 1),
                                 skip_group_check=True)
            s2 = ps.tile([128, 512], F32, tag='mm', name=f'{R}s2_{tag}')
            for db in range(DB):
                nc.tensor.matmul(s2[:], onesd_s[:], sq[:, db, :],
                                 start=(db == 0), stop=(db == 1),
                                 skip_group_check=True)
            m2 = tp.tile([128, 512], F32, tag='lns', name=f'{R}m2_{tag}')
            nc.scalar.activation(m2[:], mu[:], AF.Square)
            var = tp.tile([128, 512], F32, tag='lns', name=f'{R}var_{tag}')
            nc.vector.tensor_sub(var[:], s2[:], m2[:])
            lnv = tp.tile([128, 512], F32, tag='lnv', name=f'{R}lnv_{tag}', bufs=1)
            nc.scalar.activation(lnv[:], var[:], AF.Ln, bias=eps_s[:, 0:1])
            rdt = F32 if fin_stats is not None else BF16
            rstd = tp.tile([128, 512], rdt, tag='rstd', name=f'{R}rs_{tag}')
            nc.scalar.activation(rstd[:], lnv[:], AF.Exp, scale=-0.5)
            nm = tp.tile([128, 512], rdt, tag='nm', name=f'{R}nm_{tag}')
            nc.vector.scalar_tensor_tensor(nm[:], mu[:], -1.0, rstd[:],
                                           ALU.mult, ALU.mult)
            if fin_stats is not None:
                fin_stats.append((rstd, nm))
                return
            for db in range(DB):
                t1 = tp.tile([128, 512], BF16, tag='t1', name=f'{R}t1_{tag}{db}')
                nc.vector.tensor_mul(t1[:], rt[:, db, sl], rstd[:])
                nc.vector.tensor_add(xn_t[:, db, sl], t1[:], nm[:])

        def qk(l, b, xn, qT, kT, wqkvo_s, pvec_s):
            sl = slice(b * S, (b + 1) * S)
            for mat, (dst, bc) in enumerate([(qT, 0), (kT, 2)]):
                for mb in range(DB):
                    mp = ps.tile([128, 512], F32, tag='mm',
                                 name=f'{R}qk{l}{b}{mat}{mb}')
                    for kb in range(DB):
                        nc.tensor.matmul(
                            mp[:],
                            wqkvo_s[:, mat, kb, mb * 128:(mb + 1) * 128],
                            xn[:, kb, sl], start=(kb == 0), stop=(kb == 1),
                            skip_group_check=True)
                    nc.vector.tensor_scalar(
                        dst[:, mb, sl], mp[:],
                        pvec_s[:, bc + mb:bc + mb + 1], None, ALU.add)

        def vproj(l, b, xn, vzs, wqkvo_s, bvb_s):
            for jc in range(4):
                vp = ps.tile([128, 256], F32, tag='sc', name=f'{R}v{l}{b}{jc}')
                nc.tensor.matmul(vp[:], identb_s[:], bvb_s[:],
                                 start=True, stop=False, skip_group_check=True)
                for kb in range(DB):
                    nc.tensor.matmul(
                        vp[:],
                        xn[:, kb, b * S + jc * 128:b * S + (jc + 1) * 128],
                        wqkvo_s[:, 2, kb, :], start=False, stop=(kb == 1),
                        skip_group_check=True)
                vp_r = vp.rearrange('p (hb he pc) -> p hb he pc', hb=2, pc=64)
                for hb in range(2):
                    vz_r = vzs[hb].rearrange(
                        'p jc (he two) m -> p jc he two m', two=2)
                    for par in range(2):
                        nc.vector.tensor_copy(
                            vz_r[:, jc, :, par, 64 * par:64 * par + 32],
                            vp_r[:, hb, :, 32 * par:32 * par + 32])

        def attn_group(l, b, hb, qT, kT, vzs, estrip_s, sstrip_s):
            cz = ps.tile([128, 2, 512], F32, tag='cz', name=f'{R}cz{l}{b}{hb}')
            for jc in range(4):
                probs = []
                for hh in range(4):
                    hg = 4 * hb + hh
                    off = 511 - jc * 128
                    scp = ps.tile([128, 512], F32, tag='sc',
                                  name=f'{R}sc{l}{b}{hb}{jc}{hh}')
                    nc.tensor.matmul(
                        scp[:],
                        kT[32 * hh:32 * hh + 32, hb,
                           b * S + jc * 128:b * S + (jc + 1) * 128],
                        qT[32 * hh:32 * hh + 32, hb, b * S:(b + 1) * S],
                        start=True, stop=True,
                        tile_position=(32 * hh, 0), skip_group_check=True)
                    if hg in S_IDX:
                        pri = prp.tile([128, 512], I16, tag='pri',
                                       name=f'{R}pi{l}{b}{hb}{jc}{hh}', bufs=4)
                        nc.vector.scalar_tensor_tensor(
                            pri[:], scp[:], SCA,
                            sstrip_s[:, S_IDX[hg], off:off + 512],
                            ALU.mult, ALU.add)
                        probs.append(pri.bitcast(BF16))
                    else:
                        pr = prp.tile([128, 512], BF16, tag='pr',
                                      name=f'{R}pr{l}{b}{hb}{jc}{hh}')
                        nc.scalar.activation(pr[:], scp[:], AF.Exp)
                        eng = nc.gpsimd if jc in MULT_GPS_JC else nc.vector
                        eng.tensor_tensor(
                            pr[:], pr[:],
                            estrip_s[:, E_IDX[hg], off:off + 512], ALU.mult)
                        probs.append(pr)
                for hh in range(4):
                    nc.tensor.matmul(
                        cz[:, hh // 2, :], vzs[hb][:, jc, hh, :], probs[hh],
                        start=(jc == 0 and hh % 2 == 0),
                        stop=(jc == 3 and hh % 2 == 1),
                        skip_group_check=True)
            # normalize: Z reciprocal, partition-shift via stream_shuffle
            # (no DMA on the critical path), multiply into a bf16 SBUF tile
            # that the permuted out-projection reads directly.
            rec = tp.tile([128, 2, 512], F32, tag='rec',
                          name=f'{R}rc{l}{b}{hb}')
            nc.vector.reciprocal_approx_fast(rec[:], cz[:])
            zrec = zp.tile([128, 2, 512], F32, tag='zrec',
                           name=f'{R}zr{l}{b}{hb}')
            ident32 = list(range(32))
            nc.vector.stream_shuffle(zrec[0:32, :, :], rec[32:64, :, :],
                                     ident32)
            nc.vector.stream_shuffle(zrec[64:96, :, :], rec[96:128, :, :],
                                     ident32)
            ctxn = cnp.tile([128, 2, 512], BF16, tag='ctxn',
                            name=f'{R}cn{l}{b}{hb}')
            nc.vector.tensor_mul(ctxn[:], cz[:], zrec[:])
            return ctxn

        def outproj(l, b, ctxns, woP_s, pvec_s):
            sl = slice(b * S, (b + 1) * S)
            for mb in range(DB):
                op = ps.tile([128, 512], F32, tag='mm', name=f'{R}op{l}{b}{mb}')
                for hb in range(2):
                    for pb in range(2):
                        nc.tensor.matmul(
                            op[:], woP_s[:, hb, pb, mb * 128:(mb + 1) * 128],
                            ctxns[hb][:, pb, :],
                            start=(hb == 0 and pb == 0),
                            stop=(hb == 1 and pb == 1),
                            skip_group_check=True)
                nc.vector.scalar_tensor_tensor(
                    rt[:, mb, sl], op[:], pvec_s[:, 4 + mb:5 + mb],
                    rt[:, mb, sl], ALU.add, ALU.add)

        def ffn(l, b, xn2, wff1_s, wff2_s, pvec_s):
            sl = slice(b * S, (b + 1) * S)
            h1s = []
            for fb in range(8):
                hp_ = ps.tile([128, 512], F32, tag='mm', name=f'{R}h1{l}{b}{fb}')
                for kb in range(DB):
                    nc.tensor.matmul(
                        hp_[:], wff1_s[:, kb, fb * 128:(fb + 1) * 128],
                        xn2[:, kb, sl], start=(kb == 0), stop=(kb == 1),
                        skip_group_check=True)
                h1t = h1p.tile([128, 512], BF16, tag='h1',
                               name=f'{R}h1t{l}{b}{fb}')
                nc.scalar.activation(h1t[:], hp_[:], AF.Gelu,
                                     bias=pvec_s[:, 8 + fb:9 + fb])
                h1s.append(h1t)
            for db in range(DB):
                f2 = ps.tile([128, 512], F32, tag='mm', name=f'{R}f2{l}{b}{db}')
                for fb in range(8):
                    nc.tensor.matmul(
                        f2[:], wff2_s[:, fb, db * 128:(db + 1) * 128],
                        h1s[fb][:], start=(fb == 0), stop=(fb == 7),
                        skip_group_check=True)
                nc.vector.scalar_tensor_tensor(
                    rt[:, db, sl], f2[:], pvec_s[:, 6 + db:7 + db],
                    rt[:, db, sl], ALU.add, ALU.add)

        # ---------------- transformer layers
        if upto == 'conv':
            sink = h1p.tile([128, 256], F32, tag='h1', name=f'{R}sink')
            nc.vector.tensor_copy(sink[:], rt[:, 0, 0:256])
            nc.sync.dma_start(out[0, 0:128, :], sink[:])
            return
        for l in range(L):
            wqkvo_s = wp.tile([128, 3, DB, D], BF16, tag='wqkvo', name=f'{R}wm{l}')
            nc.sync.dma_start(wqkvo_s[:], din[f'wqkvo{l}'][:])
            woP_s = wp.tile([128, 2, 2, D], BF16, tag='woP', name=f'{R}wo{l}')
            nc.sync.dma_start(woP_s[:], din[f'woP{l}'][:])
            wff1_s = wp.tile([128, DB, FF], BF16, tag='wff1', name=f'{R}w1{l}')
            nc.sync.dma_start(wff1_s[:], din[f'wff1{l}'][:])
            wff2_s = wp.tile([128, 8, D], BF16, tag='wff2', name=f'{R}w2{l}')
            nc.sync.dma_start(wff2_s[:], din[f'wff2{l}'][:])
            bvb_s = wp.tile([128, D], BF16, tag='bvb', name=f'{R}bv{l}')
            nc.sync.dma_start(bvb_s[:], din[f'bvb{l}'][:])
            pvec_s = wp.tile([128, 16], F32, tag='pvec', name=f'{R}pv{l}')
            nc.sync.dma_start(pvec_s[:], din[f'pvec{l}'][:])
            estrip_s = wp.tile([128, NE, 1024], BF16, tag='estrip',
                               name=f'{R}es{l}', bufs=1)
            nc.sync.dma_start(estrip_s[:], din[f'estrip{l}'][:])
            sstrip_s = wp.tile([128, NS, 1024], F32, tag='sstrip',
                               name=f'{R}ss{l}', bufs=1)
            nc.sync.dma_start(sstrip_s[:], din[f'sstrip{l}'][:])

            xn = tp.tile([128, DB, TOK], BF16, tag='xn', name=f'{R}xn{l}')
            qT = ap.tile([128, DB, TOK], BF16, tag='qT', name=f'{R}qT{l}')
            kT = ap.tile([128, DB, TOK], BF16, tag='kT', name=f'{R}kT{l}')
            for b in range(BLOC):
                layernorm(b, xn, f'a{l}{b}')
            for b in range(BLOC):
                qk(l, b, xn, qT, kT, wqkvo_s, pvec_s)
            vz0 = [vzp.tile([128, 4, 4, 128], BF16, tag='vz',
                            name=f'{R}vz{l}0{hb}') for hb in range(2)]
            vproj(l, 0, xn, vz0, wqkvo_s, bvb_s)
            cn00 = attn_group(l, 0, 0, qT, kT, vz0, estrip_s, sstrip_s)
            cn01 = attn_group(l, 0, 1, qT, kT, vz0, estrip_s, sstrip_s)
            vz1 = [vzp.tile([128, 4, 4, 128], BF16, tag='vz',
                            name=f'{R}vz{l}1{hb}') for hb in range(2)]
            vproj(l, 1, xn, vz1, wqkvo_s, bvb_s)
            cn10 = attn_group(l, 1, 0, qT, kT, vz1, estrip_s, sstrip_s)
            outproj(l, 0, [cn00, cn01], woP_s, pvec_s)
            xn2 = tp.tile([128, DB, TOK], BF16, tag='xn', name=f'{R}xn2_{l}')
            layernorm(0, xn2, f'f{l}0')
            cn11 = attn_group(l, 1, 1, qT, kT, vz1, estrip_s, sstrip_s)
            outproj(l, 1, [cn10, cn11], woP_s, pvec_s)
            layernorm(1, xn2, f'f{l}1')
            ffn(l, 0, xn2, wff1_s, wff2_s, pvec_s)
            ffn(l, 1, xn2, wff1_s, wff2_s, pvec_s)

        if upto != 'full':
            sink = h1p.tile([128, 256], F32, tag='h1', name=f'{R}sink')
            nc.vector.tensor_copy(sink[:], rt[:, 0, 0:256])
            nc.sync.dma_start(out[0, 0:128, :], sink[:])
            return
        # ---------------- final LN (+affine) and transpose to token-major
        fin = tp.tile([128, DB, TOK], F32, tag='fin', name=f'{R}fin', bufs=1)
        stats = []
        for b in range(BLOC):
            layernorm(b, None, f'fin{b}', fin_stats=stats)
        for b in range(BLOC):
            sl = slice(b * S, (b + 1) * S)
            rstd, nm = stats[b]
            for db in range(DB):
                rstd_g = tp.tile([128, 512], F32, tag='rstd_g',
                                 name=f'{R}rg{b}{db}', bufs=1)
                nc.vector.tensor_scalar(rstd_g[:], rstd[:],
                                        fvec_s[:, 0 + db:1 + db], None, ALU.mult)
                nm_gb = tp.tile([128, 512], F32, tag='nm_gb', name=f'{R}ng{b}{db}', bufs=1)
                nc.vector.tensor_scalar(nm_gb[:], nm[:],
                                        fvec_s[:, 0 + db:1 + db],
                                        fvec_s[:, 2 + db:3 + db],
                                        ALU.mult, ALU.add)
                t1 = tp.tile([128, 512], F32, tag='ft1', name=f'{R}ft1{b}{db}', bufs=1)
                nc.gpsimd.tensor_tensor(t1[:], rt[:, db, sl], rstd_g[:],
                                        ALU.mult)
                nc.vector.tensor_add(fin[:, db, sl], t1[:], nm_gb[:])
        for b in range(BLOC):
            for jc in range(4):
                tc_sl = slice(b * S + jc * 128, b * S + (jc + 1) * 128)
                pst = ps.tile([128, 256], F32, tag='sc', name=f'{R}tr{b}{jc}')
                for db in range(DB):
                    nc.tensor.transpose(pst[:, db * 128:(db + 1) * 128],
                                        fin[:, db, tc_sl], identf_s[:])
                osb = h1p.tile([128, 256], F32, tag='h1', name=f'{R}ot{b}{jc}')
                nc.vector.tensor_copy(osb[:], pst[:])
                nc.sync.dma_start(out[b, jc * 128:(jc + 1) * 128, :], osb[:])

    for _rep in range(repeat):
        emit_body(f'r{_rep}_')

    for pool in [ps, cnp, zp, xip, h1p, prp, vzp, tp, ap, wp, cst]:
        pool.release()
    tc_cm.__exit__(None, None, None)
    nc.finalize()
    return nc


# ---------------------------------------------------------------- entry point
def kernel(**inputs):
    p = _prep(inputs)
    if 'nc' not in _CACHE:
        _CACHE['nc'] = _build()
    nc = _CACHE['nc']
    x = np.ascontiguousarray(np.asarray(inputs['x'], np.float32))
    in_maps = []
    for c in range(NCORES):
        m = dict(p)
        m['x'] = np.ascontiguousarray(x[c * BLOC:(c + 1) * BLOC])
        in_maps.append(m)
    res = run_bass_kernel_spmd(nc, in_maps, core_ids=list(range(NCORES)),
                               trace=TRACE)
    out = np.concatenate([r['out'] for r in res.results], axis=0)
    kernel.last_results = res
    return np.ascontiguousarray(out.astype(np.float32))


# revision 30
# speedup vs baseline: 1.0243x; 1.0243x over previous
"""Trainium2 Bass kernel for nn_ConvTranBackbone (conv tokenizer + 4-layer
transformer encoder). Data-parallel over batch: 16 batch elems -> 8 cores x 2.

V2 design (vs v1 baseline):
- bf16 weights + activations for all transformer matmuls (fast LDWEIGHTS/FWL);
  residual stream stays fp32. Conv1 fp32, conv2 bf16.
- Relative-position bias applied multiplicatively after exp (probs =
  exp(s) * exp(bias)) via a shifted bf16 strip multiply on DVE, or fused
  into a one-op DVE "Schraudolph" exp (scores*A + strip -> int16, bitcast
  bf16) for SCHRAUD_HEADS. No identity-matmul bias strips on the PE.
- LN rstd via ACT Ln->Exp (one activation table set shared with attention
  exp); DVE reciprocal replaced by reciprocal_approx_fast for softmax Z.
- Per-b stage pipelining: the two batch elements' stages are emitted
  interleaved so PE matmuls overlap the other stream's elementwise chains
  (keeps the PE HAM clock-gate warm).
- PSUM: 'mm' [128,2,512]x2 + 'sc' [128,512]x2 + 'cz' [128,2,512]x1 = 8 banks.
"""
import sys
import math

sys.path.insert(0, '/opt/trn_rl_repo')

import numpy as np
import ml_dtypes

import concourse.bass as bass
import concourse.bacc as bacc
import concourse.mybir as mybir
import concourse.tile as tile
from concourse.bass_utils import run_bass_kernel_spmd

F32 = mybir.dt.float32
F32R = mybir.dt.float32r
BF16 = mybir.dt.bfloat16
I16 = mybir.dt.int16
AF = mybir.ActivationFunctionType
ALU = mybir.AluOpType

B, C_IN, S, D, H, L, FF = 16, 32, 512, 256, 8, 4, 1024
HD = D // H          # 32
EPS = 1e-5
NCORES = 8
BLOC = B // NCORES   # 2 batch elems per core
DB = 2               # d blocks of 128
TOK = BLOC * S       # 1024 tokens per core

# Schraudolph exp constants (bf16 flavor: y = round(x*SCA + SCB) as int16,
# bitcast to bf16). The constant offset cancels in softmax normalization.
SCA = float((1 << 7) / math.log(2.0))
SCB = float(127 * 128 - 0.043 * 128)
SCHRAUD_HEADS = (2, 3, 6, 7)    # global head idx -> probs via DVE fused exp
ACT_HEADS = tuple(h for h in range(H) if h not in SCHRAUD_HEADS)
E_IDX = {h: i for i, h in enumerate(ACT_HEADS)}
S_IDX = {h: i for i, h in enumerate(SCHRAUD_HEADS)}
NE, NS = len(ACT_HEADS), len(SCHRAUD_HEADS)

# engine knobs: fraction of exp-strip multiplies routed to gpsimd, by jc
MULT_GPS_JC = (0, 2)     # jc values whose strip multiplies run on gpsimd

TRACE = False
_CACHE = {}


# ---------------------------------------------------------------- host prep
def _pos_encoding():
    pos = np.arange(S, dtype=np.float32)[:, None]
    div = np.exp(np.arange(0, D, 2, dtype=np.float32) * (-math.log(10000.0) / D))
    scale = D / S
    pe = np.zeros((S, D), dtype=np.float32)
    pe[:, 0::2] = np.sin(pos * div * scale)
    pe[:, 1::2] = np.cos(pos * div * scale)
    return pe


def _prep(inp):
    f = lambda x: np.ascontiguousarray(np.asarray(x, np.float32))
    bf = lambda x: np.ascontiguousarray(np.asarray(x, ml_dtypes.bfloat16))
    p = {}
    s1 = f(inp['bn1_g']) / np.sqrt(np.float32(1.0) + np.float32(EPS))
    b1c = f(inp['conv1_b']) * s1 + f(inp['bn1_b'])
    s2 = f(inp['bn2_g']) / np.sqrt(np.float32(1.0) + np.float32(EPS))
    b2c = f(inp['conv2_b']) * s2 + f(inp['bn2_b'])
    cvec = np.zeros((128, DB, 4), np.float32)
    for db in range(DB):
        cvec[:, db, 0] = s1[db * 128:(db + 1) * 128]
        cvec[:, db, 1] = b1c[db * 128:(db + 1) * 128]
        cvec[:, db, 2] = s2[db * 128:(db + 1) * 128]
        cvec[:, db, 3] = b2c[db * 128:(db + 1) * 128]
    p['cvec'] = cvec

    w1 = f(inp['conv1_w'])
    w1A = np.zeros((128, D), np.float32)
    for kk in range(4):
        w1A[32 * kk:32 * kk + 32, :] = w1[:, :, kk].T
    w1B = np.zeros((96, D), np.float32)
    for j in range(3):
        w1B[32 * j:32 * j + 32, :] = w1[:, :, 4 + j].T
    p['w1A'], p['w1B'] = w1A, np.ascontiguousarray(w1B)

    w2 = f(inp['conv2_w'])
    w2t = np.zeros((128, DB, 5, D), np.float32)
    for cb in range(DB):
        for k in range(5):
            w2t[:, cb, k, :] = w2[:, cb * 128:(cb + 1) * 128, k].T
    p['w2t'] = w2t.astype(ml_dtypes.bfloat16)

    pe = _pos_encoding()
    p['peT'] = np.ascontiguousarray(pe.T.reshape(DB, 128, S).transpose(1, 0, 2))

    sc = np.float32(HD ** -0.5)
    for l in range(L):
        g1, b1l = f(inp['ln1_g'][l]), f(inp['ln1_b'][l])
        g2, b2l = f(inp['ln2_g'][l]), f(inp['ln2_b'][l])
        wq = f(inp['wq'][l]) * sc
        wk, wv, wo = f(inp['wk'][l]), f(inp['wv'][l]), f(inp['wo'][l])
        wm = np.zeros((128, 3, DB, D), np.float32)
        for i, w in enumerate([g1[:, None] * wq, g1[:, None] * wk,
                               g1[:, None] * wv]):
            for kb in range(DB):
                wm[:, i, kb, :] = w[kb * 128:(kb + 1) * 128, :]
        p[f'wqkvo{l}'] = wm.astype(ml_dtypes.bfloat16)
        # out-projection weights permuted to read normalized ctx bank tiles
        # directly: ctxn[p, pb] rows 0:32 hold head 4*hb+2*pb, rows 64:96
        # head 4*hb+2*pb+1, rows 32:64/96:128 are Z junk (zero weight).
        woP = np.zeros((128, 2, 2, D), np.float32)
        for hb in range(2):
            for pb in range(2):
                d0 = 128 * hb + 64 * pb
                woP[0:32, hb, pb, :] = wo[d0:d0 + 32, :]
                woP[64:96, hb, pb, :] = wo[d0 + 32:d0 + 64, :]
        p[f'woP{l}'] = woP.astype(ml_dtypes.bfloat16)
        p[f'bvb{l}'] = np.tile((b1l @ wv)[None, :], (128, 1)).astype(
            ml_dtypes.bfloat16)
        w1f = f(inp['w1'][l])
        w1m = np.zeros((128, DB, FF), np.float32)
        w1e = g2[:, None] * w1f
        for kb in range(DB):
            w1m[:, kb, :] = w1e[kb * 128:(kb + 1) * 128, :]
        p[f'wff1{l}'] = w1m.astype(ml_dtypes.bfloat16)
        w2f = f(inp['w2'][l])
        w2m = np.zeros((128, 8, D), np.float32)
        for kb in range(8):
            w2m[:, kb, :] = w2f[kb * 128:(kb + 1) * 128, :]
        p[f'wff2{l}'] = w2m.astype(ml_dtypes.bfloat16)
        # per-partition bias pack: cols [bq(2), bk(2), bo(2), b2(2), b1(8)]
        pv = np.zeros((128, 16), np.float32)
        bq, bk = b1l @ wq, b1l @ wk
        bo, b2v = f(inp['bo'][l]), f(inp['b2'][l])
        b1e = b2l @ w1f + f(inp['b1'][l])
        for db in range(DB):
            pv[:, 0 + db] = bq[db * 128:(db + 1) * 128]
            pv[:, 2 + db] = bk[db * 128:(db + 1) * 128]
            pv[:, 4 + db] = bo[db * 128:(db + 1) * 128]
            pv[:, 6 + db] = b2v[db * 128:(db + 1) * 128]
        for fb in range(8):
            pv[:, 8 + fb] = b1e[fb * 128:(fb + 1) * 128]
        p[f'pvec{l}'] = pv
        # shifted strips: strip[p, h, c] corresponds to tab[c - p, h]
        tab = f(inp['bias_table'][l])            # [2S-1, H]
        est = np.ones((128, NE, 1024), np.float32)
        sst = np.full((128, NS, 1024), SCB, np.float32)
        for pp in range(128):
            hi = min(1024, pp + 2 * S - 1)
            for h in ACT_HEADS:
                est[pp, E_IDX[h], pp:hi] = np.exp(tab[0:hi - pp, h])
            for h in SCHRAUD_HEADS:
                sst[pp, S_IDX[h], pp:hi] = tab[0:hi - pp, h] * SCA + SCB
        p[f'estrip{l}'] = est.astype(ml_dtypes.bfloat16)
        p[f'sstrip{l}'] = sst
    fvec = np.zeros((128, 4), np.float32)
    for db in range(DB):
        fvec[:, 0 + db] = f(inp['fn_g'])[db * 128:(db + 1) * 128]
        fvec[:, 2 + db] = f(inp['fn_b'])[db * 128:(db + 1) * 128]
    p['fvec'] = fvec
    p['identb'] = np.eye(128, dtype=ml_dtypes.bfloat16)
    p['identf'] = np.eye(128, dtype=np.float32)
    p['onesd'] = np.full((128, 128), 1.0 / 256.0, np.float32)
    p['zeros16'] = np.zeros((128, 16), np.float32)
    # vz slot template: per head slot h, Z-ones at cols [64*(h%2)+32, +32)
    vzt = np.zeros((128, 4, 4, 128), np.float32)
    for h in range(4):
        par = h % 2
        vzt[:, :, h, 64 * par + 32:64 * par + 64] = 1.0
    p['vztmpl'] = vzt.astype(ml_dtypes.bfloat16)
    return p


# ---------------------------------------------------------------- device build
def _pin_act_tables(nc):
    """Steer the act-table-load pass to natural_log_exp_and_others for
    Exp/Ln/Square (it picks the first set containing each function, which
    otherwise thrashes exp_and_others <-> natural_log on every LayerNorm).
    Only set *contents* are edited, never list order, so act_func_set_id
    indices stay aligned with act_info.json."""
    from concourse.hw_specs import get_activation_tables
    tabs = get_activation_tables(nc.m.arch)
    keep = {AF.Exp, AF.Ln, AF.Square}
    for name in list(tabs):
        if name == 'natural_log_exp_and_others':
            break
        tabs[name] -= keep


def _build(repeat=1, upto='full'):
    nc = bacc.Bacc()
    _pin_act_tables(nc)
    din = {}

    def dinp(name, shape, dt=F32R):
        din[name] = nc.dram_tensor(name, list(shape), dt, kind='ExternalInput')
        return din[name]

    x = dinp('x', [BLOC, C_IN, S])
    w1A = dinp('w1A', [128, D])
    w1B = dinp('w1B', [96, D])
    w2t = dinp('w2t', [128, DB, 5, D], BF16)
    cvec = dinp('cvec', [128, DB, 4], F32)
    peT = dinp('peT', [128, DB, S], F32)
    identb = dinp('identb', [128, 128], BF16)
    identf = dinp('identf', [128, 128], F32)
    onesd = dinp('onesd', [128, 128], F32R)
    zeros16 = dinp('zeros16', [128, 16], F32R)
    vztmpl = dinp('vztmpl', [128, 4, 4, 128], BF16)
    fvec = dinp('fvec', [128, 4], F32)
    for l in range(L):
        dinp(f'wqkvo{l}', [128, 3, DB, D], BF16)
        dinp(f'woP{l}', [128, 2, 2, D], BF16)
        dinp(f'wff1{l}', [128, DB, FF], BF16)
        dinp(f'wff2{l}', [128, 8, D], BF16)
        dinp(f'bvb{l}', [128, D], BF16)
        dinp(f'pvec{l}', [128, 16], F32)
        dinp(f'estrip{l}', [128, NE, 1024], BF16)
        dinp(f'sstrip{l}', [128, NS, 1024], F32)
    out = nc.dram_tensor('out', [BLOC, S, D], F32, kind='ExternalOutput')

    tc_cm = tile.TileContext(nc)
    tc = tc_cm.__enter__()
    cst = tc.alloc_tile_pool(name='cst', bufs=1)
    wp = tc.alloc_tile_pool(name='wp', bufs=2)
    ap = tc.alloc_tile_pool(name='ap', bufs=1)
    tp = tc.alloc_tile_pool(name='tp', bufs=2)
    vzp = tc.alloc_tile_pool(name='vzp', bufs=2)
    prp = tc.alloc_tile_pool(name='prp', bufs=6)
    h1p = tc.alloc_tile_pool(name='h1p', bufs=12)
    xip = tc.alloc_tile_pool(name='xip', bufs=4)
    zp = tc.alloc_tile_pool(name='zp', bufs=2)
    cnp = tc.alloc_tile_pool(name='cnp', bufs=4)
    ps = tc.alloc_tile_pool(name='ps', bufs=2, space='PSUM')

    # ---- consts
    identb_s = cst.tile([128, 128], BF16)
    nc.sync.dma_start(identb_s[:], identb[:])
    identf_s = cst.tile([128, 128], F32)
    nc.sync.dma_start(identf_s[:], identf[:])
    onesd_s = cst.tile([128, 128], F32R)
    nc.sync.dma_start(onesd_s[:], onesd[:])
    z16_s = cst.tile([128, 16], F32R)
    nc.sync.dma_start(z16_s[:], zeros16[:])
    cvec_s = cst.tile([128, DB, 4], F32)
    nc.sync.dma_start(cvec_s[:], cvec[:])
    fvec_s = cst.tile([128, 4], F32)
    nc.sync.dma_start(fvec_s[:], fvec[:])
    peT_s = cst.tile([128, DB, S], F32)
    nc.sync.dma_start(peT_s[:], peT[:])
    eps_s = cst.tile([128, 1], F32)
    nc.vector.memset(eps_s[:], EPS)
    w1A_s = cst.tile([128, D], F32R)
    nc.sync.dma_start(w1A_s[:], w1A[:])
    w1B_s = cst.tile([96, D], F32R)
    nc.sync.dma_start(w1B_s[:], w1B[:])
    w2t_s = cst.tile([128, DB, 5, D], BF16)
    nc.sync.dma_start(w2t_s[:], w2t[:])

    # vz slot templates (zeros + Z-ones). v columns are rewritten per use;
    # the static template regions persist across pool-slot reuse.
    for i in range(2):
        vzt_t = vzp.tile([128, 4, 4, 128], BF16, tag='vz', name=f'vzi{i}')
        nc.sync.dma_start(vzt_t[:], vztmpl[:])
    # zrec slots: Z-reciprocal rows are stream-shuffled in per use; the junk
    # rows (32:64, 96:128) are memset once so the out-of-band lanes of the
    # normalize multiply stay finite (their woP rows are zero).
    for i in range(2):
        zr_t = zp.tile([128, 2, 512], F32, tag='zrec', name=f'zri{i}')
        nc.vector.memset(zr_t[32:64, :, :], 1.0)
        nc.vector.memset(zr_t[96:128, :, :], 1.0)

    # persistent residual stream, feature-major [d mod 128, d//128, token]
    rt = ap.tile([128, DB, TOK], F32R)

    def emit_body(R):
        # ---------------- conv tokenizer (conv1 fp32, conv2 bf16)
        xts = {}
        for b in range(BLOC):
            X4 = xip.tile([128, 512], F32R, tag='xi', name=f'{R}x4_{b}')
            nc.sync.dma_start(X4[0:32, 3:512], x[b, :, 0:509])
            nc.sync.dma_start(X4[32:64, 2:512], x[b, :, 0:510])
            nc.sync.dma_start(X4[64:96, 1:512], x[b, :, 0:511])
            nc.sync.dma_start(X4[96:128, 0:512], x[b, :, 0:512])
            nc.sync.dma_start(X4[0:32, 0:3], z16_s[0:32, 0:3])
            nc.sync.dma_start(X4[32:64, 0:2], z16_s[32:64, 0:2])
            nc.sync.dma_start(X4[64:96, 0:1], z16_s[64:96, 0:1])
            X3 = xip.tile([128, 512], F32R, tag='xi', name=f'{R}x3_{b}')
            nc.sync.dma_start(X3[0:32, 0:511], x[b, :, 1:512])
            nc.sync.dma_start(X3[32:64, 0:510], x[b, :, 2:512])
            nc.sync.dma_start(X3[64:96, 0:509], x[b, :, 3:512])
            nc.sync.dma_start(X3[0:32, 511:512], z16_s[0:32, 0:1])
            nc.sync.dma_start(X3[32:64, 510:512], z16_s[32:64, 0:2])
            nc.sync.dma_start(X3[64:96, 509:512], z16_s[64:96, 0:3])
            xts[b] = (X4, X3)
        for b in range(BLOC):
            X4, X3 = xts[b]
            hp = tp.tile([128, DB, 516], BF16, tag='hp', name=f'{R}hp_{b}')
            c1 = ps.tile([128, 2, 512], F32, tag='sc', name=f'{R}c1_{b}')
            for dc in range(DB):
                nc.tensor.matmul(c1[:, dc, :], w1A_s[:, dc * 128:(dc + 1) * 128],
                                 X4[:], start=True, stop=False,
                                 skip_group_check=True)
                nc.tensor.matmul(c1[:, dc, :], w1B_s[:, dc * 128:(dc + 1) * 128],
                                 X3[0:96, :], start=False, stop=True,
                                 skip_group_check=True)
                nc.gpsimd.memset(hp[:, dc, 0:2], 0.0)
                nc.gpsimd.memset(hp[:, dc, 514:516], 0.0)
                nc.scalar.activation(hp[:, dc, 2:514], c1[:, dc, :], AF.Gelu,
                                     bias=cvec_s[:, dc, 1:2],
                                     scale=cvec_s[:, dc, 0:1])
            c2 = ps.tile([128, 2, 512], F32, tag='sc', name=f'{R}c2_{b}')
            for dc in range(DB):
                for cb in range(DB):
                    for k in range(5):
                        nc.tensor.matmul(
                            c2[:, dc, :], w2t_s[:, cb, k, dc * 128:(dc + 1) * 128],
                            hp[:, cb, k:k + 512],
                            start=(cb == 0 and k == 0),
                            stop=(cb == 1 and k == 4), skip_group_check=True)
                tg = h1p.tile([128, 512], BF16, tag='h1', name=f'{R}tg_{b}_{dc}')
                nc.scalar.activation(tg[:], c2[:, dc, :], AF.Gelu,
                                     bias=cvec_s[:, dc, 3:4],
                                     scale=cvec_s[:, dc, 2:3])
                nc.vector.tensor_add(rt[:, dc, b * S:(b + 1) * S],
                                     tg[:], peT_s[:, dc, :])

        # ---------------- per-(layer, b) stage emitters
        def layernorm(b, xn_t, tag, fin_stats=None):
            sl = slice(b * S, (b + 1) * S)
            sq = tp.tile([128, DB, 512], F32R, tag='sq', name=f'{R}sq_{tag}', bufs=1)
            nc.gpsimd.tensor_tensor(sq[:], rt[:, :, sl], rt[:, :, sl], ALU.mult)
            st = ps.tile([128, 2, 512], F32, tag='sc', name=f'{R}st_{tag}')
            for db in range(DB):
                nc.tensor.matmul(st[:, 0, :], onesd_s[:], rt[:, db, sl],
                                 start=(db == 0), stop=(db == 1),
                                 skip_group_check=True)
            for db in range(DB):
                nc.tensor.matmul(st[:, 1, :], onesd_s[:], sq[:, db, :],
                                 start=(db == 0), stop=(db == 1),
                                 skip_group_check=True)
            m2 = tp.tile([128, 512], F32, tag='lns', name=f'{R}m2_{tag}')
            nc.scalar.activation(m2[:], st[:, 0, :], AF.Square)
            var = tp.tile([128, 512], F32, tag='lns', name=f'{R}var_{tag}')
            nc.vector.tensor_sub(var[:], st[:, 1, :], m2[:])
            lnv = tp.tile([128, 512], F32, tag='lnv', name=f'{R}lnv_{tag}', bufs=1)
            nc.scalar.activation(lnv[:], var[:], AF.Ln, bias=eps_s[:, 0:1])
            rstd = tp.tile([128, 512], F32, tag='rstd', name=f'{R}rs_{tag}')
            nc.scalar.activation(rstd[:], lnv[:], AF.Exp, scale=-0.5)
            nmdt = F32 if fin_stats is not None else BF16
            nm = tp.tile([128, 512], nmdt, tag='nm', name=f'{R}nm_{tag}')
            nc.vector.scalar_tensor_tensor(nm[:], st[:, 0, :], -1.0, rstd[:],
                                           ALU.mult, ALU.mult)
            if fin_stats is not None:
                fin_stats.append((rstd, nm))
                return
            for db in range(DB):
                t1 = tp.tile([128, 512], BF16, tag='t1', name=f'{R}t1_{tag}{db}')
                nc.gpsimd.tensor_tensor(t1[:], rt[:, db, sl], rstd[:], ALU.mult)
                nc.vector.tensor_add(xn_t[:, db, sl], t1[:], nm[:])

        def qk(l, b, xn, qT, kT, wqkvo_s, pvec_s):
            sl = slice(b * S, (b + 1) * S)
            for mat, (dst, bc) in enumerate([(qT, 0), (kT, 2)]):
                mp = ps.tile([128, 2, 512], F32, tag='sc',
                             name=f'{R}qk{l}{b}{mat}')
                for mb in range(DB):
                    for kb in range(DB):
                        nc.tensor.matmul(
                            mp[:, mb, :],
                            wqkvo_s[:, mat, kb, mb * 128:(mb + 1) * 128],
                            xn[:, kb, sl], start=(kb == 0), stop=(kb == 1),
                            skip_group_check=True)
                for mb in range(DB):
                    nc.scalar.activation(
                        dst[:, mb, sl], mp[:, mb, :], AF.Identity,
                        bias=pvec_s[:, bc + mb:bc + mb + 1])

        def vproj(l, b, xn, vzs, wqkvo_s, bvb_s):
            for jc in range(4):
                vp = ps.tile([128, 256], F32, tag='sc', name=f'{R}v{l}{b}{jc}')
                nc.tensor.matmul(vp[:], identb_s[:], bvb_s[:],
                                 start=True, stop=False, skip_group_check=True)
                for kb in range(DB):
                    nc.tensor.matmul(
                        vp[:],
                        xn[:, kb, b * S + jc * 128:b * S + (jc + 1) * 128],
                        wqkvo_s[:, 2, kb, :], start=False, stop=(kb == 1),
                        skip_group_check=True)
                vp_r = vp.rearrange('p (hb he pc) -> p hb he pc', hb=2, pc=64)
                for hb in range(2):
                    vz_r = vzs[hb].rearrange(
                        'p jc (he two) m -> p jc he two m', two=2)
                    for par in range(2):
                        nc.vector.tensor_copy(
                            vz_r[:, jc, :, par, 64 * par:64 * par + 32],
                            vp_r[:, hb, :, 32 * par:32 * par + 32])

        def attn_group(l, b, hb, qT, kT, vzs, estrip_s, sstrip_s):
            cz = ps.tile([128, 2, 512], F32, tag='cz', name=f'{R}cz{l}{b}{hb}')
            for jc in range(4):
                off = 511 - jc * 128
                # head pairs: (0,1) -> ACT exp + strip mult, (2,3) -> fused
                # Schraudolph exp on DVE (SCHRAUD_HEADS covers hh 2,3).
                scA = ps.tile([128, 2, 512], F32, tag='sc',
                              name=f'{R}scA{l}{b}{hb}{jc}')
                scB = ps.tile([128, 2, 512], F32, tag='sc',
                              name=f'{R}scB{l}{b}{hb}{jc}')
                for hh in range(4):
                    dstp = scA if hh < 2 else scB
                    nc.tensor.matmul(
                        dstp[:, hh % 2, :],
                        kT[32 * hh:32 * hh + 32, hb,
                           b * S + jc * 128:b * S + (jc + 1) * 128],
                        qT[32 * hh:32 * hh + 32, hb, b * S:(b + 1) * S],
                        start=True, stop=True,
                        tile_position=(32 * hh, 0), skip_group_check=True)
                prA = prp.tile([128, 2, 512], BF16, tag='pr',
                               name=f'{R}pr{l}{b}{hb}{jc}')
                nc.scalar.activation(prA[:], scA[:], AF.Exp)
                eng = nc.gpsimd if jc in MULT_GPS_JC else nc.vector
                eng.tensor_tensor(
                    prA[:], prA[:],
                    estrip_s[:, 2 * hb:2 * hb + 2, off:off + 512], ALU.mult)
                prB = prp.tile([128, 2, 512], I16, tag='pri',
                               name=f'{R}pi{l}{b}{hb}{jc}', bufs=4)
                nc.vector.scalar_tensor_tensor(
                    prB[:], scB[:], SCA,
                    sstrip_s[:, 2 * hb:2 * hb + 2, off:off + 512],
                    ALU.mult, ALU.add)
                prBb = prB.bitcast(BF16)
                probs = [prA[:, 0, :], prA[:, 1, :], prBb[:, 0, :],
                         prBb[:, 1, :]]
                for hh in range(4):
                    nc.tensor.matmul(
                        cz[:, hh // 2, :], vzs[hb][:, jc, hh, :], probs[hh],
                        start=(jc == 0 and hh % 2 == 0),
                        stop=(jc == 3 and hh % 2 == 1),
                        skip_group_check=True)
            # normalize: Z reciprocal, partition-shift via stream_shuffle
            # (no DMA on the critical path), multiply into a bf16 SBUF tile
            # that the permuted out-projection reads directly.
            rec = tp.tile([128, 2, 512], F32, tag='rec',
                          name=f'{R}rc{l}{b}{hb}')
            nc.vector.reciprocal_approx_fast(rec[:], cz[:])
            zrec = zp.tile([128, 2, 512], F32, tag='zrec',
                           name=f'{R}zr{l}{b}{hb}')
            ident32 = list(range(32))
            nc.vector.stream_shuffle(zrec[0:32, :, :], rec[32:64, :, :],
                                     ident32)
            nc.vector.stream_shuffle(zrec[64:96, :, :], rec[96:128, :, :],
                                     ident32)
            ctxn = cnp.tile([128, 2, 512], BF16, tag='ctxn',
                            name=f'{R}cn{l}{b}{hb}')
            nc.vector.tensor_mul(ctxn[:], cz[:], zrec[:])
            return ctxn

        def outproj(l, b, ctxns, woP_s, pvec_s):
            sl = slice(b * S, (b + 1) * S)
            op = ps.tile([128, 2, 512], F32, tag='sc', name=f'{R}op{l}{b}')
            for mb in range(DB):
                for hb in range(2):
                    for pb in range(2):
                        nc.tensor.matmul(
                            op[:, mb, :],
                            woP_s[:, hb, pb, mb * 128:(mb + 1) * 128],
                            ctxns[hb][:, pb, :],
                            start=(hb == 0 and pb == 0),
                            stop=(hb == 1 and pb == 1),
                            skip_group_check=True)
            for mb in range(DB):
                nc.vector.scalar_tensor_tensor(
                    rt[:, mb, sl], op[:, mb, :], pvec_s[:, 4 + mb:5 + mb],
                    rt[:, mb, sl], ALU.add, ALU.add)

        def ffn(l, b, xn2, wff1_s, wff2_s, pvec_s):
            sl = slice(b * S, (b + 1) * S)
            h1s = []
            for fp in range(4):
                hp_ = ps.tile([128, 2, 512], F32, tag='sc',
                              name=f'{R}h1{l}{b}{fp}')
                for half in range(2):
                    fb = 2 * fp + half
                    for kb in range(DB):
                        nc.tensor.matmul(
                            hp_[:, half, :],
                            wff1_s[:, kb, fb * 128:(fb + 1) * 128],
                            xn2[:, kb, sl], start=(kb == 0), stop=(kb == 1),
                            skip_group_check=True)
                for half in range(2):
                    fb = 2 * fp + half
                    h1t = h1p.tile([128, 512], BF16, tag='h1',
                                   name=f'{R}h1t{l}{b}{fb}')
                    nc.scalar.activation(h1t[:], hp_[:, half, :], AF.Gelu,
                                         bias=pvec_s[:, 8 + fb:9 + fb])
                    h1s.append(h1t)
            f2 = ps.tile([128, 2, 512], F32, tag='sc', name=f'{R}f2{l}{b}')
            for db in range(DB):
                for fb in range(8):
                    nc.tensor.matmul(
                        f2[:, db, :], wff2_s[:, fb, db * 128:(db + 1) * 128],
                        h1s[fb][:], start=(fb == 0), stop=(fb == 7),
                        skip_group_check=True)
            for db in range(DB):
                nc.vector.scalar_tensor_tensor(
                    rt[:, db, sl], f2[:, db, :], pvec_s[:, 6 + db:7 + db],
                    rt[:, db, sl], ALU.add, ALU.add)

        # ---------------- transformer layers
        if upto == 'conv':
            sink = h1p.tile([128, 256], F32, tag='h1', name=f'{R}sink')
            nc.vector.tensor_copy(sink[:], rt[:, 0, 0:256])
            nc.sync.dma_start(out[0, 0:128, :], sink[:])
            return
        for l in range(L):
            wqkvo_s = wp.tile([128, 3, DB, D], BF16, tag='wqkvo', name=f'{R}wm{l}')
            nc.sync.dma_start(wqkvo_s[:], din[f'wqkvo{l}'][:])
            woP_s = wp.tile([128, 2, 2, D], BF16, tag='woP', name=f'{R}wo{l}')
            nc.sync.dma_start(woP_s[:], din[f'woP{l}'][:])
            wff1_s = wp.tile([128, DB, FF], BF16, tag='wff1', name=f'{R}w1{l}')
            nc.sync.dma_start(wff1_s[:], din[f'wff1{l}'][:])
            wff2_s = wp.tile([128, 8, D], BF16, tag='wff2', name=f'{R}w2{l}')
            nc.sync.dma_start(wff2_s[:], din[f'wff2{l}'][:])
            bvb_s = wp.tile([128, D], BF16, tag='bvb', name=f'{R}bv{l}')
            nc.sync.dma_start(bvb_s[:], din[f'bvb{l}'][:])
            pvec_s = wp.tile([128, 16], F32, tag='pvec', name=f'{R}pv{l}')
            nc.sync.dma_start(pvec_s[:], din[f'pvec{l}'][:])
            estrip_s = wp.tile([128, NE, 1024], BF16, tag='estrip',
                               name=f'{R}es{l}', bufs=1)
            nc.sync.dma_start(estrip_s[:], din[f'estrip{l}'][:])
            sstrip_s = wp.tile([128, NS, 1024], F32, tag='sstrip',
                               name=f'{R}ss{l}', bufs=1)
            nc.sync.dma_start(sstrip_s[:], din[f'sstrip{l}'][:])

            xn = tp.tile([128, DB, TOK], BF16, tag='xn', name=f'{R}xn{l}')
            qT = ap.tile([128, DB, TOK], BF16, tag='qT', name=f'{R}qT{l}')
            kT = ap.tile([128, DB, TOK], BF16, tag='kT', name=f'{R}kT{l}')
            for b in range(BLOC):
                layernorm(b, xn, f'a{l}{b}')
            for b in range(BLOC):
                qk(l, b, xn, qT, kT, wqkvo_s, pvec_s)
            vz0 = [vzp.tile([128, 4, 4, 128], BF16, tag='vz',
                            name=f'{R}vz{l}0{hb}') for hb in range(2)]
            vproj(l, 0, xn, vz0, wqkvo_s, bvb_s)
            cn00 = attn_group(l, 0, 0, qT, kT, vz0, estrip_s, sstrip_s)
            cn01 = attn_group(l, 0, 1, qT, kT, vz0, estrip_s, sstrip_s)
            vz1 = [vzp.tile([128, 4, 4, 128], BF16, tag='vz',
                            name=f'{R}vz{l}1{hb}') for hb in range(2)]
            vproj(l, 1, xn, vz1, wqkvo_s, bvb_s)
            cn10 = attn_group(l, 1, 0, qT, kT, vz1, estrip_s, sstrip_s)
            outproj(l, 0, [cn00, cn01], woP_s, pvec_s)
            xn2 = tp.tile([128, DB, TOK], BF16, tag='xn', name=f'{R}xn2_{l}')
            layernorm(0, xn2, f'f{l}0')
            cn11 = attn_group(l, 1, 1, qT, kT, vz1, estrip_s, sstrip_s)
            outproj(l, 1, [cn10, cn11], woP_s, pvec_s)
            layernorm(1, xn2, f'f{l}1')
            ffn(l, 0, xn2, wff1_s, wff2_s, pvec_s)
            ffn(l, 1, xn2, wff1_s, wff2_s, pvec_s)

        if upto != 'full':
            sink = h1p.tile([128, 256], F32, tag='h1', name=f'{R}sink')
            nc.vector.tensor_copy(sink[:], rt[:, 0, 0:256])
            nc.sync.dma_start(out[0, 0:128, :], sink[:])
            return
        # ---------------- final LN (+affine) and transpose to token-major
        fin = tp.tile([128, DB, TOK], F32, tag='fin', name=f'{R}fin', bufs=1)
        stats = []
        for b in range(BLOC):
            layernorm(b, None, f'fin{b}', fin_stats=stats)
        for b in range(BLOC):
            sl = slice(b * S, (b + 1) * S)
            rstd, nm = stats[b]
            for db in range(DB):
                rstd_g = tp.tile([128, 512], F32, tag='rstd_g',
                                 name=f'{R}rg{b}{db}', bufs=1)
                nc.vector.tensor_scalar(rstd_g[:], rstd[:],
                                        fvec_s[:, 0 + db:1 + db], None, ALU.mult)
                nm_gb = tp.tile([128, 512], F32, tag='nm_gb', name=f'{R}ng{b}{db}', bufs=1)
                nc.vector.tensor_scalar(nm_gb[:], nm[:],
                                        fvec_s[:, 0 + db:1 + db],
                                        fvec_s[:, 2 + db:3 + db],
                                        ALU.mult, ALU.add)
                t1 = tp.tile([128, 512], F32, tag='ft1', name=f'{R}ft1{b}{db}', bufs=1)
                nc.gpsimd.tensor_tensor(t1[:], rt[:, db, sl], rstd_g[:],
                                        ALU.mult)
                nc.vector.tensor_add(fin[:, db, sl], t1[:], nm_gb[:])
        for b in range(BLOC):
            for jc in range(4):
                tc_sl = slice(b * S + jc * 128, b * S + (jc + 1) * 128)
                pst = ps.tile([128, 256], F32, tag='sc', name=f'{R}tr{b}{jc}')
                for db in range(DB):
                    nc.tensor.transpose(pst[:, db * 128:(db + 1) * 128],
                                        fin[:, db, tc_sl], identf_s[:])
                osb = h1p.tile([128, 256], F32, tag='h1', name=f'{R}ot{b}{jc}')
                nc.vector.tensor_copy(osb[:], pst[:])
                nc.sync.dma_start(out[b, jc * 128:(jc + 1) * 128, :], osb[:])

    for _rep in range(repeat):
        emit_body(f'r{_rep}_')

    for pool in [ps, cnp, zp, xip, h1p, prp, vzp, tp, ap, wp, cst]:
        pool.release()
    tc_cm.__exit__(None, None, None)
    nc.finalize()
    return nc


# ---------------------------------------------------------------- entry point
def kernel(**inputs):
    p = _prep(inputs)
    if 'nc' not in _CACHE:
        _CACHE['nc'] = _build()
    nc = _CACHE['nc']
    x = np.ascontiguousarray(np.asarray(inputs['x'], np.float32))
    in_maps = []
    for c in range(NCORES):
        m = dict(p)
        m['x'] = np.ascontiguousarray(x[c * BLOC:(c + 1) * BLOC])
        in_maps.append(m)
    res = run_bass_kernel_spmd(nc, in_maps, core_ids=list(range(NCORES)),
                               trace=TRACE)
    out = np.concatenate([r['out'] for r in res.results], axis=0)
    kernel.last_results = res
    return np.ascontiguousarray(out.astype(np.float32))


# revision 33
# speedup vs baseline: 1.1034x; 1.0772x over previous
"""Trainium2 Bass kernel for nn_ConvTranBackbone (conv tokenizer + 4-layer
transformer encoder). Data-parallel over batch: 16 batch elems -> 8 cores x 2.

V2 design (vs v1 baseline):
- bf16 weights + activations for all transformer matmuls (fast LDWEIGHTS/FWL);
  residual stream stays fp32. Conv1 fp32, conv2 bf16.
- Relative-position bias applied multiplicatively after exp (probs =
  exp(s) * exp(bias)) via a shifted bf16 strip multiply on DVE, or fused
  into a one-op DVE "Schraudolph" exp (scores*A + strip -> int16, bitcast
  bf16) for SCHRAUD_HEADS. No identity-matmul bias strips on the PE.
- LN rstd via ACT Ln->Exp (one activation table set shared with attention
  exp); DVE reciprocal replaced by reciprocal_approx_fast for softmax Z.
- Per-b stage pipelining: the two batch elements' stages are emitted
  interleaved so PE matmuls overlap the other stream's elementwise chains
  (keeps the PE HAM clock-gate warm).
- PSUM: 'mm' [128,2,512]x2 + 'sc' [128,512]x2 + 'cz' [128,2,512]x1 = 8 banks.
"""
import sys
import math

sys.path.insert(0, '/opt/trn_rl_repo')

import numpy as np
import ml_dtypes

import concourse.bass as bass
import concourse.bacc as bacc
import concourse.mybir as mybir
import concourse.tile as tile
from concourse.bass_utils import run_bass_kernel_spmd

F32 = mybir.dt.float32
F32R = mybir.dt.float32r
BF16 = mybir.dt.bfloat16
I16 = mybir.dt.int16
AF = mybir.ActivationFunctionType
ALU = mybir.AluOpType

B, C_IN, S, D, H, L, FF = 16, 32, 512, 256, 8, 4, 1024
HD = D // H          # 32
EPS = 1e-5
NCORES = 8
BLOC = B // NCORES   # 2 batch elems per core
DB = 2               # d blocks of 128
TOK = BLOC * S       # 1024 tokens per core

# Schraudolph exp constants (bf16 flavor: y = round(x*SCA + SCB) as int16,
# bitcast to bf16). The constant offset cancels in softmax normalization.
SCA = float((1 << 7) / math.log(2.0))
SCB = float(127 * 128 - 0.043 * 128)
SCHRAUD_HEADS = (2, 3, 6, 7)    # global head idx -> probs via DVE fused exp
ACT_HEADS = tuple(h for h in range(H) if h not in SCHRAUD_HEADS)
E_IDX = {h: i for i, h in enumerate(ACT_HEADS)}
S_IDX = {h: i for i, h in enumerate(SCHRAUD_HEADS)}
NE, NS = len(ACT_HEADS), len(SCHRAUD_HEADS)

# engine knobs: fraction of exp-strip multiplies routed to gpsimd, by jc
MULT_GPS_JC = (0, 2)     # jc values whose strip multiplies run on gpsimd

TRACE = False
_CACHE = {}


# ---------------------------------------------------------------- host prep
def _pos_encoding():
    pos = np.arange(S, dtype=np.float32)[:, None]
    div = np.exp(np.arange(0, D, 2, dtype=np.float32) * (-math.log(10000.0) / D))
    scale = D / S
    pe = np.zeros((S, D), dtype=np.float32)
    pe[:, 0::2] = np.sin(pos * div * scale)
    pe[:, 1::2] = np.cos(pos * div * scale)
    return pe


def _prep(inp):
    f = lambda x: np.ascontiguousarray(np.asarray(x, np.float32))
    bf = lambda x: np.ascontiguousarray(np.asarray(x, ml_dtypes.bfloat16))
    p = {}
    s1 = f(inp['bn1_g']) / np.sqrt(np.float32(1.0) + np.float32(EPS))
    b1c = f(inp['conv1_b']) * s1 + f(inp['bn1_b'])
    s2 = f(inp['bn2_g']) / np.sqrt(np.float32(1.0) + np.float32(EPS))
    b2c = f(inp['conv2_b']) * s2 + f(inp['bn2_b'])
    cvec = np.zeros((128, DB, 4), np.float32)
    for db in range(DB):
        cvec[:, db, 0] = s1[db * 128:(db + 1) * 128]
        cvec[:, db, 1] = b1c[db * 128:(db + 1) * 128]
        cvec[:, db, 2] = s2[db * 128:(db + 1) * 128]
        cvec[:, db, 3] = b2c[db * 128:(db + 1) * 128]
    p['cvec'] = cvec

    w1 = f(inp['conv1_w'])
    w1A = np.zeros((128, D), np.float32)
    for kk in range(4):
        w1A[32 * kk:32 * kk + 32, :] = w1[:, :, kk].T
    w1B = np.zeros((96, D), np.float32)
    for j in range(3):
        w1B[32 * j:32 * j + 32, :] = w1[:, :, 4 + j].T
    p['w1A'], p['w1B'] = w1A, np.ascontiguousarray(w1B)

    w2 = f(inp['conv2_w'])
    w2t = np.zeros((128, DB, 5, D), np.float32)
    for cb in range(DB):
        for k in range(5):
            w2t[:, cb, k, :] = w2[:, cb * 128:(cb + 1) * 128, k].T
    p['w2t'] = w2t.astype(ml_dtypes.bfloat16)

    pe = _pos_encoding()
    p['peT'] = np.ascontiguousarray(pe.T.reshape(DB, 128, S).transpose(1, 0, 2))

    sc = np.float32(HD ** -0.5)
    for l in range(L):
        g1, b1l = f(inp['ln1_g'][l]), f(inp['ln1_b'][l])
        g2, b2l = f(inp['ln2_g'][l]), f(inp['ln2_b'][l])
        wq = f(inp['wq'][l]) * sc
        wk, wv, wo = f(inp['wk'][l]), f(inp['wv'][l]), f(inp['wo'][l])
        wm = np.zeros((128, 3, DB, D), np.float32)
        for i, w in enumerate([g1[:, None] * wq, g1[:, None] * wk,
                               g1[:, None] * wv]):
            for kb in range(DB):
                wm[:, i, kb, :] = w[kb * 128:(kb + 1) * 128, :]
        p[f'wqkvo{l}'] = wm.astype(ml_dtypes.bfloat16)
        # out-projection weights permuted to read normalized ctx bank tiles
        # directly: ctxn[p, pb] rows 0:32 hold head 4*hb+2*pb, rows 64:96
        # head 4*hb+2*pb+1, rows 32:64/96:128 are Z junk (zero weight).
        woP = np.zeros((128, 2, 2, D), np.float32)
        for hb in range(2):
            for pb in range(2):
                d0 = 128 * hb + 64 * pb
                woP[0:32, hb, pb, :] = wo[d0:d0 + 32, :]
                woP[64:96, hb, pb, :] = wo[d0 + 32:d0 + 64, :]
        p[f'woP{l}'] = woP.astype(ml_dtypes.bfloat16)
        p[f'bvb{l}'] = np.tile((b1l @ wv)[None, :], (128, 1)).astype(
            ml_dtypes.bfloat16)
        w1f = f(inp['w1'][l])
        w1m = np.zeros((128, DB, FF), np.float32)
        w1e = g2[:, None] * w1f
        for kb in range(DB):
            w1m[:, kb, :] = w1e[kb * 128:(kb + 1) * 128, :]
        p[f'wff1{l}'] = w1m.astype(ml_dtypes.bfloat16)
        w2f = f(inp['w2'][l])
        w2m = np.zeros((128, 8, D), np.float32)
        for kb in range(8):
            w2m[:, kb, :] = w2f[kb * 128:(kb + 1) * 128, :]
        p[f'wff2{l}'] = w2m.astype(ml_dtypes.bfloat16)
        # per-partition bias pack: cols [bq(2), bk(2), bo(2), b2(2), b1(8)]
        pv = np.zeros((128, 16), np.float32)
        bq, bk = b1l @ wq, b1l @ wk
        bo, b2v = f(inp['bo'][l]), f(inp['b2'][l])
        b1e = b2l @ w1f + f(inp['b1'][l])
        for db in range(DB):
            pv[:, 0 + db] = bq[db * 128:(db + 1) * 128]
            pv[:, 2 + db] = bk[db * 128:(db + 1) * 128]
            pv[:, 4 + db] = bo[db * 128:(db + 1) * 128]
            pv[:, 6 + db] = b2v[db * 128:(db + 1) * 128]
        for fb in range(8):
            pv[:, 8 + fb] = b1e[fb * 128:(fb + 1) * 128]
        p[f'pvec{l}'] = pv
        # shifted strips: strip[p, h, c] corresponds to tab[c - p, h]
        tab = f(inp['bias_table'][l])            # [2S-1, H]
        est = np.ones((128, NE, 1024), np.float32)
        sst = np.full((128, NS, 1024), SCB, np.float32)
        for pp in range(128):
            hi = min(1024, pp + 2 * S - 1)
            for h in ACT_HEADS:
                est[pp, E_IDX[h], pp:hi] = np.exp(tab[0:hi - pp, h])
            for h in SCHRAUD_HEADS:
                sst[pp, S_IDX[h], pp:hi] = tab[0:hi - pp, h] * SCA + SCB
        p[f'estrip{l}'] = est.astype(ml_dtypes.bfloat16)
        p[f'sstrip{l}'] = sst
    fvec = np.zeros((128, 4), np.float32)
    for db in range(DB):
        fvec[:, 0 + db] = f(inp['fn_g'])[db * 128:(db + 1) * 128]
        fvec[:, 2 + db] = f(inp['fn_b'])[db * 128:(db + 1) * 128]
    p['fvec'] = fvec
    p['identb'] = np.eye(128, dtype=ml_dtypes.bfloat16)
    p['identf'] = np.eye(128, dtype=np.float32)
    p['onesd'] = np.full((128, 128), 1.0 / 256.0, np.float32)
    p['zeros16'] = np.zeros((128, 16), np.float32)
    # vz slot template: per head slot h, Z-ones at cols [64*(h%2)+32, +32)
    vzt = np.zeros((128, 4, 4, 128), np.float32)
    for h in range(4):
        par = h % 2
        vzt[:, :, h, 64 * par + 32:64 * par + 64] = 1.0
    p['vztmpl'] = vzt.astype(ml_dtypes.bfloat16)
    return p


# ---------------------------------------------------------------- device build
def _pin_act_tables(nc):
    """Steer the act-table-load pass to natural_log_exp_and_others for
    Exp/Ln/Square (it picks the first set containing each function, which
    otherwise thrashes exp_and_others <-> natural_log on every LayerNorm).
    Only set *contents* are edited, never list order, so act_func_set_id
    indices stay aligned with act_info.json."""
    from concourse.hw_specs import get_activation_tables
    tabs = get_activation_tables(nc.m.arch)
    keep = {AF.Exp, AF.Ln, AF.Square}
    for name in list(tabs):
        if name == 'natural_log_exp_and_others':
            break
        tabs[name] -= keep


def _build(repeat=1, upto='full'):
    nc = bacc.Bacc()
    _pin_act_tables(nc)
    din = {}

    def dinp(name, shape, dt=F32R):
        din[name] = nc.dram_tensor(name, list(shape), dt, kind='ExternalInput')
        return din[name]

    x = dinp('x', [BLOC, C_IN, S])
    w1A = dinp('w1A', [128, D])
    w1B = dinp('w1B', [96, D])
    w2t = dinp('w2t', [128, DB, 5, D], BF16)
    cvec = dinp('cvec', [128, DB, 4], F32)
    peT = dinp('peT', [128, DB, S], F32)
    identb = dinp('identb', [128, 128], BF16)
    identf = dinp('identf', [128, 128], F32)
    onesd = dinp('onesd', [128, 128], F32R)
    zeros16 = dinp('zeros16', [128, 16], F32R)
    vztmpl = dinp('vztmpl', [128, 4, 4, 128], BF16)
    fvec = dinp('fvec', [128, 4], F32)
    for l in range(L):
        dinp(f'wqkvo{l}', [128, 3, DB, D], BF16)
        dinp(f'woP{l}', [128, 2, 2, D], BF16)
        dinp(f'wff1{l}', [128, DB, FF], BF16)
        dinp(f'wff2{l}', [128, 8, D], BF16)
        dinp(f'bvb{l}', [128, D], BF16)
        dinp(f'pvec{l}', [128, 16], F32)
        dinp(f'estrip{l}', [128, NE, 1024], BF16)
        dinp(f'sstrip{l}', [128, NS, 1024], F32)
    out = nc.dram_tensor('out', [BLOC, S, D], F32, kind='ExternalOutput')

    tc_cm = tile.TileContext(nc)
    tc = tc_cm.__enter__()
    cst = tc.alloc_tile_pool(name='cst', bufs=1)
    wp = tc.alloc_tile_pool(name='wp', bufs=2)
    ap = tc.alloc_tile_pool(name='ap', bufs=1)
    tp = tc.alloc_tile_pool(name='tp', bufs=2)
    vzp = tc.alloc_tile_pool(name='vzp', bufs=2)
    prp = tc.alloc_tile_pool(name='prp', bufs=6)
    h1p = tc.alloc_tile_pool(name='h1p', bufs=10)
    xip = tc.alloc_tile_pool(name='xip', bufs=4)
    zp = tc.alloc_tile_pool(name='zp', bufs=2)
    cnp = tc.alloc_tile_pool(name='cnp', bufs=4)
    ps = tc.alloc_tile_pool(name='ps', bufs=2, space='PSUM')

    # ---- consts
    identb_s = cst.tile([128, 128], BF16)
    nc.sync.dma_start(identb_s[:], identb[:])
    identf_s = cst.tile([128, 128], F32)
    nc.sync.dma_start(identf_s[:], identf[:])
    onesd_s = cst.tile([128, 128], F32R)
    nc.sync.dma_start(onesd_s[:], onesd[:])
    z16_s = cst.tile([128, 16], F32R)
    nc.sync.dma_start(z16_s[:], zeros16[:])
    cvec_s = cst.tile([128, DB, 4], F32)
    nc.sync.dma_start(cvec_s[:], cvec[:])
    fvec_s = cst.tile([128, 4], F32)
    nc.sync.dma_start(fvec_s[:], fvec[:])
    peT_s = cst.tile([128, DB, S], F32)
    nc.sync.dma_start(peT_s[:], peT[:])
    eps_s = cst.tile([128, 1], F32)
    nc.vector.memset(eps_s[:], EPS)
    w1A_s = cst.tile([128, D], F32R)
    nc.sync.dma_start(w1A_s[:], w1A[:])
    w1B_s = cst.tile([96, D], F32R)
    nc.sync.dma_start(w1B_s[:], w1B[:])
    w2t_s = cst.tile([128, DB, 5, D], BF16)
    nc.sync.dma_start(w2t_s[:], w2t[:])

    # vz slot templates (zeros + Z-ones). v columns are rewritten per use;
    # the static template regions persist across pool-slot reuse.
    for i in range(2):
        vzt_t = vzp.tile([128, 4, 4, 128], BF16, tag='vz', name=f'vzi{i}')
        nc.sync.dma_start(vzt_t[:], vztmpl[:])
    # zrec slots: Z-reciprocal rows are stream-shuffled in per use; the junk
    # rows (32:64, 96:128) are memset once so the out-of-band lanes of the
    # normalize multiply stay finite (their woP rows are zero).
    for i in range(2):
        zr_t = zp.tile([128, 2, 512], F32, tag='zrec', name=f'zri{i}')
        nc.vector.memset(zr_t[32:64, :, :], 1.0)
        nc.vector.memset(zr_t[96:128, :, :], 1.0)

    # persistent residual stream, feature-major [d mod 128, d//128, token]
    rt = ap.tile([128, DB, TOK], F32R)

    def emit_body(R):
        # ---------------- conv tokenizer (conv1 fp32, conv2 bf16)
        xts = {}
        for b in range(BLOC):
            X4 = xip.tile([128, 512], F32R, tag='xi', name=f'{R}x4_{b}')
            nc.sync.dma_start(X4[0:32, 3:512], x[b, :, 0:509])
            nc.sync.dma_start(X4[32:64, 2:512], x[b, :, 0:510])
            nc.sync.dma_start(X4[64:96, 1:512], x[b, :, 0:511])
            nc.sync.dma_start(X4[96:128, 0:512], x[b, :, 0:512])
            nc.sync.dma_start(X4[0:32, 0:3], z16_s[0:32, 0:3])
            nc.sync.dma_start(X4[32:64, 0:2], z16_s[32:64, 0:2])
            nc.sync.dma_start(X4[64:96, 0:1], z16_s[64:96, 0:1])
            X3 = xip.tile([128, 512], F32R, tag='xi', name=f'{R}x3_{b}')
            nc.sync.dma_start(X3[0:32, 0:511], x[b, :, 1:512])
            nc.sync.dma_start(X3[32:64, 0:510], x[b, :, 2:512])
            nc.sync.dma_start(X3[64:96, 0:509], x[b, :, 3:512])
            nc.sync.dma_start(X3[0:32, 511:512], z16_s[0:32, 0:1])
            nc.sync.dma_start(X3[32:64, 510:512], z16_s[32:64, 0:2])
            nc.sync.dma_start(X3[64:96, 509:512], z16_s[64:96, 0:3])
            xts[b] = (X4, X3)
        for b in range(BLOC):
            X4, X3 = xts[b]
            hp = tp.tile([128, DB, 516], BF16, tag='hp', name=f'{R}hp_{b}')
            c1 = ps.tile([128, 2, 512], F32, tag='sc', name=f'{R}c1_{b}')
            for dc in range(DB):
                nc.tensor.matmul(c1[:, dc, :], w1A_s[:, dc * 128:(dc + 1) * 128],
                                 X4[:], start=True, stop=False,
                                 skip_group_check=True)
                nc.tensor.matmul(c1[:, dc, :], w1B_s[:, dc * 128:(dc + 1) * 128],
                                 X3[0:96, :], start=False, stop=True,
                                 skip_group_check=True)
                nc.gpsimd.memset(hp[:, dc, 0:2], 0.0)
                nc.gpsimd.memset(hp[:, dc, 514:516], 0.0)
                nc.scalar.activation(hp[:, dc, 2:514], c1[:, dc, :], AF.Gelu,
                                     bias=cvec_s[:, dc, 1:2],
                                     scale=cvec_s[:, dc, 0:1])
            c2 = ps.tile([128, 2, 512], F32, tag='sc', name=f'{R}c2_{b}')
            for dc in range(DB):
                for cb in range(DB):
                    for k in range(5):
                        nc.tensor.matmul(
                            c2[:, dc, :], w2t_s[:, cb, k, dc * 128:(dc + 1) * 128],
                            hp[:, cb, k:k + 512],
                            start=(cb == 0 and k == 0),
                            stop=(cb == 1 and k == 4), skip_group_check=True)
                tg = h1p.tile([128, 512], BF16, tag='h1', name=f'{R}tg_{b}_{dc}')
                nc.scalar.activation(tg[:], c2[:, dc, :], AF.Gelu,
                                     bias=cvec_s[:, dc, 3:4],
                                     scale=cvec_s[:, dc, 2:3])
                nc.vector.tensor_add(rt[:, dc, b * S:(b + 1) * S],
                                     tg[:], peT_s[:, dc, :])

        # ---------------- per-(layer, b) stage emitters
        def layernorm(b, xn_t, tag, fin_stats=None):
            sl = slice(b * S, (b + 1) * S)
            sq = tp.tile([128, DB, 512], F32R, tag='sq', name=f'{R}sq_{tag}', bufs=1)
            nc.gpsimd.tensor_tensor(sq[:], rt[:, :, sl], rt[:, :, sl], ALU.mult)
            st = ps.tile([128, 2, 512], F32, tag='sc', name=f'{R}st_{tag}')
            for db in range(DB):
                nc.tensor.matmul(st[:, 0, :], onesd_s[:], rt[:, db, sl],
                                 start=(db == 0), stop=(db == 1),
                                 skip_group_check=True)
            for db in range(DB):
                nc.tensor.matmul(st[:, 1, :], onesd_s[:], sq[:, db, :],
                                 start=(db == 0), stop=(db == 1),
                                 skip_group_check=True)
            m2 = tp.tile([128, 512], F32, tag='lns', name=f'{R}m2_{tag}')
            nc.scalar.activation(m2[:], st[:, 0, :], AF.Square)
            var = tp.tile([128, 512], F32, tag='lns', name=f'{R}var_{tag}')
            nc.vector.tensor_sub(var[:], st[:, 1, :], m2[:])
            lnv = tp.tile([128, 512], F32, tag='lnv', name=f'{R}lnv_{tag}', bufs=1)
            nc.scalar.activation(lnv[:], var[:], AF.Ln, bias=eps_s[:, 0:1])
            rstd = tp.tile([128, 512], F32, tag='rstd', name=f'{R}rs_{tag}')
            nc.scalar.activation(rstd[:], lnv[:], AF.Exp, scale=-0.5)
            nmdt = F32 if fin_stats is not None else BF16
            nm = tp.tile([128, 512], nmdt, tag='nm', name=f'{R}nm_{tag}')
            nc.vector.scalar_tensor_tensor(nm[:], st[:, 0, :], -1.0, rstd[:],
                                           ALU.mult, ALU.mult)
            if fin_stats is not None:
                fin_stats.append((rstd, nm))
                return
            for db in range(DB):
                t1 = tp.tile([128, 512], BF16, tag='t1', name=f'{R}t1_{tag}{db}')
                nc.gpsimd.tensor_tensor(t1[:], rt[:, db, sl], rstd[:], ALU.mult)
                nc.vector.tensor_add(xn_t[:, db, sl], t1[:], nm[:])

        def qk(l, b, xn, qT, kT, wqkvo_s, pvec_s):
            sl = slice(b * S, (b + 1) * S)
            for mat, (dst, bc) in enumerate([(qT, 0), (kT, 2)]):
                mp = ps.tile([128, 2, 512], F32, tag='sc',
                             name=f'{R}qk{l}{b}{mat}')
                for mb in range(DB):
                    for kb in range(DB):
                        nc.tensor.matmul(
                            mp[:, mb, :],
                            wqkvo_s[:, mat, kb, mb * 128:(mb + 1) * 128],
                            xn[:, kb, sl], start=(kb == 0), stop=(kb == 1),
                            skip_group_check=True)
                for mb in range(DB):
                    nc.scalar.activation(
                        dst[:, mb, sl], mp[:, mb, :], AF.Identity,
                        bias=pvec_s[:, bc + mb:bc + mb + 1])

        def vproj(l, b, xn, vzs, wqkvo_s, bvb_s):
            for jc in range(4):
                vp = ps.tile([128, 256], F32, tag='sc', name=f'{R}v{l}{b}{jc}')
                nc.tensor.matmul(vp[:], identb_s[:], bvb_s[:],
                                 start=True, stop=False, skip_group_check=True)
                for kb in range(DB):
                    nc.tensor.matmul(
                        vp[:],
                        xn[:, kb, b * S + jc * 128:b * S + (jc + 1) * 128],
                        wqkvo_s[:, 2, kb, :], start=False, stop=(kb == 1),
                        skip_group_check=True)
                vp_r = vp.rearrange('p (hb he pc) -> p hb he pc', hb=2, pc=64)
                for hb in range(2):
                    vz_r = vzs[hb].rearrange(
                        'p jc (he two) m -> p jc he two m', two=2)
                    for par in range(2):
                        nc.vector.tensor_copy(
                            vz_r[:, jc, :, par, 64 * par:64 * par + 32],
                            vp_r[:, hb, :, 32 * par:32 * par + 32])

        def attn_pair(l, b, qT, kT, vzs, estrip_s, sstrip_s):
            """Both hb groups of one batch element, jc-interleaved so the PE
            fills one group's probs latency with the other group's matmuls."""
            czs = [ps.tile([128, 2, 512], F32, tag='cz',
                           name=f'{R}cz{l}{b}{hb}') for hb in range(2)]
            scs = {}
            for jc in range(4):
                off = 511 - jc * 128
                for hb in range(2):
                    scA = ps.tile([128, 2, 512], F32, tag='sc',
                                  name=f'{R}scA{l}{b}{hb}{jc}')
                    scB = ps.tile([128, 2, 512], F32, tag='sc',
                                  name=f'{R}scB{l}{b}{hb}{jc}')
                    scs[hb] = (scA, scB)
                    for hh in range(4):
                        dstp = scA if hh < 2 else scB
                        nc.tensor.matmul(
                            dstp[:, hh % 2, :],
                            kT[32 * hh:32 * hh + 32, hb,
                               b * S + jc * 128:b * S + (jc + 1) * 128],
                            qT[32 * hh:32 * hh + 32, hb, b * S:(b + 1) * S],
                            start=True, stop=True,
                            tile_position=(32 * hh, 0), skip_group_check=True)
                for hb in range(2):
                    scA, scB = scs[hb]
                    prA = prp.tile([128, 2, 512], BF16, tag='pr',
                                   name=f'{R}pr{l}{b}{hb}{jc}', bufs=2)
                    nc.scalar.activation(prA[:], scA[:], AF.Exp)
                    prm = prp.tile([128, 2, 512], BF16, tag='prm',
                                   name=f'{R}pm{l}{b}{hb}{jc}', bufs=2)
                    eng = nc.gpsimd if jc in MULT_GPS_JC else nc.vector
                    eng.tensor_tensor(
                        prm[:], prA[:],
                        estrip_s[:, 2 * hb:2 * hb + 2, off:off + 512],
                        ALU.mult)
                    prB = prp.tile([128, 2, 512], I16, tag='pri',
                                   name=f'{R}pi{l}{b}{hb}{jc}', bufs=4)
                    nc.vector.scalar_tensor_tensor(
                        prB[:], scB[:], SCA,
                        sstrip_s[:, 2 * hb:2 * hb + 2, off:off + 512],
                        ALU.mult, ALU.add)
                    prBb = prB.bitcast(BF16)
                    probs = [prm[:, 0, :], prm[:, 1, :], prBb[:, 0, :],
                             prBb[:, 1, :]]
                    for hh in range(4):
                        nc.tensor.matmul(
                            czs[hb][:, hh // 2, :], vzs[hb][:, jc, hh, :],
                            probs[hh],
                            start=(jc == 0 and hh % 2 == 0),
                            stop=(jc == 3 and hh % 2 == 1),
                            skip_group_check=True)
            # normalize both groups
            ctxns = []
            ident32 = list(range(32))
            for hb in range(2):
                cz = czs[hb]
                rec = tp.tile([128, 2, 512], F32, tag='rec',
                              name=f'{R}rc{l}{b}{hb}')
                nc.vector.reciprocal_approx_fast(rec[:], cz[:])
                zrec = zp.tile([128, 2, 512], F32, tag='zrec',
                               name=f'{R}zr{l}{b}{hb}')
                nc.vector.stream_shuffle(zrec[0:32, :, :], rec[32:64, :, :],
                                         ident32)
                nc.vector.stream_shuffle(zrec[64:96, :, :], rec[96:128, :, :],
                                         ident32)
                ctxn = cnp.tile([128, 2, 512], BF16, tag='ctxn',
                                name=f'{R}cn{l}{b}{hb}')
                nc.vector.tensor_mul(ctxn[:], cz[:], zrec[:])
                ctxns.append(ctxn)
            return ctxns

        def outproj(l, b, ctxns, woP_s, pvec_s):
            sl = slice(b * S, (b + 1) * S)
            op = ps.tile([128, 2, 512], F32, tag='sc', name=f'{R}op{l}{b}')
            for mb in range(DB):
                for hb in range(2):
                    for pb in range(2):
                        nc.tensor.matmul(
                            op[:, mb, :],
                            woP_s[:, hb, pb, mb * 128:(mb + 1) * 128],
                            ctxns[hb][:, pb, :],
                            start=(hb == 0 and pb == 0),
                            stop=(hb == 1 and pb == 1),
                            skip_group_check=True)
            for mb in range(DB):
                nc.vector.scalar_tensor_tensor(
                    rt[:, mb, sl], op[:, mb, :], pvec_s[:, 4 + mb:5 + mb],
                    rt[:, mb, sl], ALU.add, ALU.add)

        def ffn(l, b, xn2, wff1_s, wff2_s, pvec_s):
            sl = slice(b * S, (b + 1) * S)
            h1s = []
            for fp in range(4):
                hp_ = ps.tile([128, 2, 512], F32, tag='sc',
                              name=f'{R}h1{l}{b}{fp}')
                for half in range(2):
                    fb = 2 * fp + half
                    for kb in range(DB):
                        nc.tensor.matmul(
                            hp_[:, half, :],
                            wff1_s[:, kb, fb * 128:(fb + 1) * 128],
                            xn2[:, kb, sl], start=(kb == 0), stop=(kb == 1),
                            skip_group_check=True)
                for half in range(2):
                    fb = 2 * fp + half
                    h1t = h1p.tile([128, 512], BF16, tag='h1',
                                   name=f'{R}h1t{l}{b}{fb}')
                    nc.scalar.activation(h1t[:], hp_[:, half, :], AF.Gelu,
                                         bias=pvec_s[:, 8 + fb:9 + fb])
                    h1s.append(h1t)
            f2 = ps.tile([128, 2, 512], F32, tag='sc', name=f'{R}f2{l}{b}')
            for db in range(DB):
                for fb in range(8):
                    nc.tensor.matmul(
                        f2[:, db, :], wff2_s[:, fb, db * 128:(db + 1) * 128],
                        h1s[fb][:], start=(fb == 0), stop=(fb == 7),
                        skip_group_check=True)
            for db in range(DB):
                nc.vector.scalar_tensor_tensor(
                    rt[:, db, sl], f2[:, db, :], pvec_s[:, 6 + db:7 + db],
                    rt[:, db, sl], ALU.add, ALU.add)

        # ---------------- transformer layers
        if upto == 'conv':
            sink = h1p.tile([128, 256], F32, tag='h1', name=f'{R}sink')
            nc.vector.tensor_copy(sink[:], rt[:, 0, 0:256])
            nc.sync.dma_start(out[0, 0:128, :], sink[:])
            return
        for l in range(L):
            wqkvo_s = wp.tile([128, 3, DB, D], BF16, tag='wqkvo', name=f'{R}wm{l}')
            nc.sync.dma_start(wqkvo_s[:], din[f'wqkvo{l}'][:])
            woP_s = wp.tile([128, 2, 2, D], BF16, tag='woP', name=f'{R}wo{l}')
            nc.sync.dma_start(woP_s[:], din[f'woP{l}'][:])
            wff1_s = wp.tile([128, DB, FF], BF16, tag='wff1', name=f'{R}w1{l}')
            nc.sync.dma_start(wff1_s[:], din[f'wff1{l}'][:])
            wff2_s = wp.tile([128, 8, D], BF16, tag='wff2', name=f'{R}w2{l}')
            nc.sync.dma_start(wff2_s[:], din[f'wff2{l}'][:])
            bvb_s = wp.tile([128, D], BF16, tag='bvb', name=f'{R}bv{l}')
            nc.sync.dma_start(bvb_s[:], din[f'bvb{l}'][:])
            pvec_s = wp.tile([128, 16], F32, tag='pvec', name=f'{R}pv{l}')
            nc.sync.dma_start(pvec_s[:], din[f'pvec{l}'][:])
            estrip_s = wp.tile([128, NE, 1024], BF16, tag='estrip',
                               name=f'{R}es{l}', bufs=1)
            nc.sync.dma_start(estrip_s[:], din[f'estrip{l}'][:])
            sstrip_s = wp.tile([128, NS, 1024], F32, tag='sstrip',
                               name=f'{R}ss{l}', bufs=1)
            nc.sync.dma_start(sstrip_s[:], din[f'sstrip{l}'][:])

            xn = tp.tile([128, DB, TOK], BF16, tag='xn', name=f'{R}xn{l}')
            qT = ap.tile([128, DB, TOK], BF16, tag='qT', name=f'{R}qT{l}')
            kT = ap.tile([128, DB, TOK], BF16, tag='kT', name=f'{R}kT{l}')
            for b in range(BLOC):
                layernorm(b, xn, f'a{l}{b}')
            for b in range(BLOC):
                qk(l, b, xn, qT, kT, wqkvo_s, pvec_s)
            vz0 = [vzp.tile([128, 4, 4, 128], BF16, tag='vz',
                            name=f'{R}vz{l}0{hb}') for hb in range(2)]
            vproj(l, 0, xn, vz0, wqkvo_s, bvb_s)
            cns0 = attn_pair(l, 0, qT, kT, vz0, estrip_s, sstrip_s)
            vz1 = [vzp.tile([128, 4, 4, 128], BF16, tag='vz',
                            name=f'{R}vz{l}1{hb}') for hb in range(2)]
            vproj(l, 1, xn, vz1, wqkvo_s, bvb_s)
            cns1 = attn_pair(l, 1, qT, kT, vz1, estrip_s, sstrip_s)
            outproj(l, 0, cns0, woP_s, pvec_s)
            xn2 = tp.tile([128, DB, TOK], BF16, tag='xn', name=f'{R}xn2_{l}')
            layernorm(0, xn2, f'f{l}0')
            outproj(l, 1, cns1, woP_s, pvec_s)
            layernorm(1, xn2, f'f{l}1')
            ffn(l, 0, xn2, wff1_s, wff2_s, pvec_s)
            ffn(l, 1, xn2, wff1_s, wff2_s, pvec_s)

        if upto != 'full':
            sink = h1p.tile([128, 256], F32, tag='h1', name=f'{R}sink')
            nc.vector.tensor_copy(sink[:], rt[:, 0, 0:256])
            nc.sync.dma_start(out[0, 0:128, :], sink[:])
            return
        # ---------------- final LN (+affine) and transpose to token-major
        fin = tp.tile([128, DB, TOK], F32, tag='fin', name=f'{R}fin', bufs=1)
        stats = []
        for b in range(BLOC):
            layernorm(b, None, f'fin{b}', fin_stats=stats)
        for b in range(BLOC):
            sl = slice(b * S, (b + 1) * S)
            rstd, nm = stats[b]
            for db in range(DB):
                rstd_g = tp.tile([128, 512], F32, tag='rstd_g',
                                 name=f'{R}rg{b}{db}', bufs=1)
                nc.vector.tensor_scalar(rstd_g[:], rstd[:],
                                        fvec_s[:, 0 + db:1 + db], None, ALU.mult)
                nm_gb = tp.tile([128, 512], F32, tag='nm_gb', name=f'{R}ng{b}{db}', bufs=1)
                nc.vector.tensor_scalar(nm_gb[:], nm[:],
                                        fvec_s[:, 0 + db:1 + db],
                                        fvec_s[:, 2 + db:3 + db],
                                        ALU.mult, ALU.add)
                t1 = tp.tile([128, 512], F32, tag='ft1', name=f'{R}ft1{b}{db}', bufs=1)
                nc.gpsimd.tensor_tensor(t1[:], rt[:, db, sl], rstd_g[:],
                                        ALU.mult)
                nc.vector.tensor_add(fin[:, db, sl], t1[:], nm_gb[:])
        for b in range(BLOC):
            for jc in range(4):
                tc_sl = slice(b * S + jc * 128, b * S + (jc + 1) * 128)
                pst = ps.tile([128, 256], F32, tag='sc', name=f'{R}tr{b}{jc}')
                for db in range(DB):
                    nc.tensor.transpose(pst[:, db * 128:(db + 1) * 128],
                                        fin[:, db, tc_sl], identf_s[:])
                osb = h1p.tile([128, 256], F32, tag='h1', name=f'{R}ot{b}{jc}')
                nc.vector.tensor_copy(osb[:], pst[:])
                nc.sync.dma_start(out[b, jc * 128:(jc + 1) * 128, :], osb[:])

    for _rep in range(repeat):
        emit_body(f'r{_rep}_')

    for pool in [ps, cnp, zp, xip, h1p, prp, vzp, tp, ap, wp, cst]:
        pool.release()
    tc_cm.__exit__(None, None, None)
    nc.finalize()
    return nc


# ---------------------------------------------------------------- entry point
def kernel(**inputs):
    p = _prep(inputs)
    if 'nc' not in _CACHE:
        _CACHE['nc'] = _build()
    nc = _CACHE['nc']
    x = np.ascontiguousarray(np.asarray(inputs['x'], np.float32))
    in_maps = []
    for c in range(NCORES):
        m = dict(p)
        m['x'] = np.ascontiguousarray(x[c * BLOC:(c + 1) * BLOC])
        in_maps.append(m)
    res = run_bass_kernel_spmd(nc, in_maps, core_ids=list(range(NCORES)),
                               trace=TRACE)
    out = np.concatenate([r['out'] for r in res.results], axis=0)
    kernel.last_results = res
    return np.ascontiguousarray(out.astype(np.float32))


# revision 35
# speedup vs baseline: 1.1126x; 1.0084x over previous
"""Trainium2 Bass kernel for nn_ConvTranBackbone (conv tokenizer + 4-layer
transformer encoder). Data-parallel over batch: 16 batch elems -> 8 cores x 2.

V2 design (vs v1 baseline):
- bf16 weights + activations for all transformer matmuls (fast LDWEIGHTS/FWL);
  residual stream stays fp32. Conv1 fp32, conv2 bf16.
- Relative-position bias applied multiplicatively after exp (probs =
  exp(s) * exp(bias)) via a shifted bf16 strip multiply on DVE, or fused
  into a one-op DVE "Schraudolph" exp (scores*A + strip -> int16, bitcast
  bf16) for SCHRAUD_HEADS. No identity-matmul bias strips on the PE.
- LN rstd via ACT Ln->Exp (one activation table set shared with attention
  exp); DVE reciprocal replaced by reciprocal_approx_fast for softmax Z.
- Per-b stage pipelining: the two batch elements' stages are emitted
  interleaved so PE matmuls overlap the other stream's elementwise chains
  (keeps the PE HAM clock-gate warm).
- PSUM: 'mm' [128,2,512]x2 + 'sc' [128,512]x2 + 'cz' [128,2,512]x1 = 8 banks.
"""
import sys
import math

sys.path.insert(0, '/opt/trn_rl_repo')

import numpy as np
import ml_dtypes

import concourse.bass as bass
import concourse.bacc as bacc
import concourse.mybir as mybir
import concourse.tile as tile
from concourse.bass_utils import run_bass_kernel_spmd

F32 = mybir.dt.float32
F32R = mybir.dt.float32r
BF16 = mybir.dt.bfloat16
I16 = mybir.dt.int16
AF = mybir.ActivationFunctionType
ALU = mybir.AluOpType

B, C_IN, S, D, H, L, FF = 16, 32, 512, 256, 8, 4, 1024
HD = D // H          # 32
EPS = 1e-5
NCORES = 8
BLOC = B // NCORES   # 2 batch elems per core
DB = 2               # d blocks of 128
TOK = BLOC * S       # 1024 tokens per core

# Schraudolph exp constants (bf16 flavor: y = round(x*SCA + SCB) as int16,
# bitcast to bf16). The constant offset cancels in softmax normalization.
SCA = float((1 << 7) / math.log(2.0))
SCB = float(127 * 128 - 0.043 * 128)
SCHRAUD_HEADS = (2, 3, 6, 7)    # global head idx -> probs via DVE fused exp
ACT_HEADS = tuple(h for h in range(H) if h not in SCHRAUD_HEADS)
E_IDX = {h: i for i, h in enumerate(ACT_HEADS)}
S_IDX = {h: i for i, h in enumerate(SCHRAUD_HEADS)}
NE, NS = len(ACT_HEADS), len(SCHRAUD_HEADS)

# engine knobs: fraction of exp-strip multiplies routed to gpsimd, by jc
MULT_GPS_JC = (0, 2)     # jc values whose strip multiplies run on gpsimd

TRACE = False
_CACHE = {}


# ---------------------------------------------------------------- host prep
def _pos_encoding():
    pos = np.arange(S, dtype=np.float32)[:, None]
    div = np.exp(np.arange(0, D, 2, dtype=np.float32) * (-math.log(10000.0) / D))
    scale = D / S
    pe = np.zeros((S, D), dtype=np.float32)
    pe[:, 0::2] = np.sin(pos * div * scale)
    pe[:, 1::2] = np.cos(pos * div * scale)
    return pe


def _prep(inp):
    f = lambda x: np.ascontiguousarray(np.asarray(x, np.float32))
    bf = lambda x: np.ascontiguousarray(np.asarray(x, ml_dtypes.bfloat16))
    p = {}
    s1 = f(inp['bn1_g']) / np.sqrt(np.float32(1.0) + np.float32(EPS))
    b1c = f(inp['conv1_b']) * s1 + f(inp['bn1_b'])
    s2 = f(inp['bn2_g']) / np.sqrt(np.float32(1.0) + np.float32(EPS))
    b2c = f(inp['conv2_b']) * s2 + f(inp['bn2_b'])
    cvec = np.zeros((128, DB, 4), np.float32)
    for db in range(DB):
        cvec[:, db, 0] = s1[db * 128:(db + 1) * 128]
        cvec[:, db, 1] = b1c[db * 128:(db + 1) * 128]
        cvec[:, db, 2] = s2[db * 128:(db + 1) * 128]
        cvec[:, db, 3] = b2c[db * 128:(db + 1) * 128]
    p['cvec'] = cvec

    w1 = f(inp['conv1_w'])
    w1A = np.zeros((128, D), np.float32)
    for kk in range(4):
        w1A[32 * kk:32 * kk + 32, :] = w1[:, :, kk].T
    w1B = np.zeros((96, D), np.float32)
    for j in range(3):
        w1B[32 * j:32 * j + 32, :] = w1[:, :, 4 + j].T
    p['w1A'] = w1A.astype(ml_dtypes.bfloat16)
    p['w1B'] = np.ascontiguousarray(w1B).astype(ml_dtypes.bfloat16)

    w2 = f(inp['conv2_w'])
    w2t = np.zeros((128, DB, 5, D), np.float32)
    for cb in range(DB):
        for k in range(5):
            w2t[:, cb, k, :] = w2[:, cb * 128:(cb + 1) * 128, k].T
    p['w2t'] = w2t.astype(ml_dtypes.bfloat16)

    pe = _pos_encoding()
    p['peT'] = np.ascontiguousarray(pe.T.reshape(DB, 128, S).transpose(1, 0, 2))

    sc = np.float32(HD ** -0.5)
    for l in range(L):
        g1, b1l = f(inp['ln1_g'][l]), f(inp['ln1_b'][l])
        g2, b2l = f(inp['ln2_g'][l]), f(inp['ln2_b'][l])
        wq = f(inp['wq'][l]) * sc
        wk, wv, wo = f(inp['wk'][l]), f(inp['wv'][l]), f(inp['wo'][l])
        wm = np.zeros((128, 3, DB, D), np.float32)
        for i, w in enumerate([g1[:, None] * wq, g1[:, None] * wk,
                               g1[:, None] * wv]):
            for kb in range(DB):
                wm[:, i, kb, :] = w[kb * 128:(kb + 1) * 128, :]
        p[f'wqkvo{l}'] = wm.astype(ml_dtypes.bfloat16)
        # out-projection weights permuted to read normalized ctx bank tiles
        # directly: ctxn[p, pb] rows 0:32 hold head 4*hb+2*pb, rows 64:96
        # head 4*hb+2*pb+1, rows 32:64/96:128 are Z junk (zero weight).
        woP = np.zeros((128, 2, 2, D), np.float32)
        for hb in range(2):
            for pb in range(2):
                d0 = 128 * hb + 64 * pb
                woP[0:32, hb, pb, :] = wo[d0:d0 + 32, :]
                woP[64:96, hb, pb, :] = wo[d0 + 32:d0 + 64, :]
        p[f'woP{l}'] = woP.astype(ml_dtypes.bfloat16)
        p[f'bvb{l}'] = np.tile((b1l @ wv)[None, :], (128, 1)).astype(
            ml_dtypes.bfloat16)
        w1f = f(inp['w1'][l])
        w1m = np.zeros((128, DB, FF), np.float32)
        w1e = g2[:, None] * w1f
        for kb in range(DB):
            w1m[:, kb, :] = w1e[kb * 128:(kb + 1) * 128, :]
        p[f'wff1{l}'] = w1m.astype(ml_dtypes.bfloat16)
        w2f = f(inp['w2'][l])
        w2m = np.zeros((128, 8, D), np.float32)
        for kb in range(8):
            w2m[:, kb, :] = w2f[kb * 128:(kb + 1) * 128, :]
        p[f'wff2{l}'] = w2m.astype(ml_dtypes.bfloat16)
        # per-partition bias pack: cols [bq(2), bk(2), bo(2), b2(2), b1(8)]
        pv = np.zeros((128, 16), np.float32)
        bq, bk = b1l @ wq, b1l @ wk
        bo, b2v = f(inp['bo'][l]), f(inp['b2'][l])
        b1e = b2l @ w1f + f(inp['b1'][l])
        for db in range(DB):
            pv[:, 0 + db] = bq[db * 128:(db + 1) * 128]
            pv[:, 2 + db] = bk[db * 128:(db + 1) * 128]
            pv[:, 4 + db] = bo[db * 128:(db + 1) * 128]
            pv[:, 6 + db] = b2v[db * 128:(db + 1) * 128]
        for fb in range(8):
            pv[:, 8 + fb] = b1e[fb * 128:(fb + 1) * 128]
        p[f'pvec{l}'] = pv
        # shifted strips: strip[p, h, c] corresponds to tab[c - p, h]
        tab = f(inp['bias_table'][l])            # [2S-1, H]
        est = np.ones((128, NE, 1032), np.float32)
        sst = np.full((128, NS, 1032), SCB, np.float32)
        for pp in range(128):
            lo, hi = pp + 1, min(1032, pp + 2 * S)
            for h in ACT_HEADS:
                est[pp, E_IDX[h], lo:hi] = np.exp(tab[0:hi - lo, h])
            for h in SCHRAUD_HEADS:
                sst[pp, S_IDX[h], lo:hi] = tab[0:hi - lo, h] * SCA + SCB
        p[f'estrip{l}'] = est.astype(ml_dtypes.bfloat16)
        p[f'sstrip{l}'] = sst
    fvec = np.zeros((128, 4), np.float32)
    for db in range(DB):
        fvec[:, 0 + db] = f(inp['fn_g'])[db * 128:(db + 1) * 128]
        fvec[:, 2 + db] = f(inp['fn_b'])[db * 128:(db + 1) * 128]
    p['fvec'] = fvec
    p['identb'] = np.eye(128, dtype=ml_dtypes.bfloat16)
    p['identf'] = np.eye(128, dtype=np.float32)
    p['onesd'] = np.full((128, 128), 1.0 / 256.0, np.float32)
    p['zeros16'] = np.zeros((128, 16), ml_dtypes.bfloat16)
    # vz slot template: per head slot h, Z-ones at cols [64*(h%2)+32, +32)
    vzt = np.zeros((128, 4, 4, 128), np.float32)
    for h in range(4):
        par = h % 2
        vzt[:, :, h, 64 * par + 32:64 * par + 64] = 1.0
    p['vztmpl'] = vzt.astype(ml_dtypes.bfloat16)
    return p


# ---------------------------------------------------------------- device build
def _pin_act_tables(nc):
    """Steer the act-table-load pass to natural_log_exp_and_others for
    Exp/Ln/Square (it picks the first set containing each function, which
    otherwise thrashes exp_and_others <-> natural_log on every LayerNorm).
    Only set *contents* are edited, never list order, so act_func_set_id
    indices stay aligned with act_info.json."""
    from concourse.hw_specs import get_activation_tables
    tabs = get_activation_tables(nc.m.arch)
    keep = {AF.Exp, AF.Ln, AF.Square}
    for name in list(tabs):
        if name == 'natural_log_exp_and_others':
            break
        tabs[name] -= keep


def _build(repeat=1, upto='full'):
    nc = bacc.Bacc()
    _pin_act_tables(nc)
    din = {}

    def dinp(name, shape, dt=F32R):
        din[name] = nc.dram_tensor(name, list(shape), dt, kind='ExternalInput')
        return din[name]

    x = dinp('x', [BLOC, C_IN, S], BF16)
    w1A = dinp('w1A', [128, D], BF16)
    w1B = dinp('w1B', [96, D], BF16)
    w2t = dinp('w2t', [128, DB, 5, D], BF16)
    cvec = dinp('cvec', [128, DB, 4], F32)
    peT = dinp('peT', [128, DB, S], F32)
    identb = dinp('identb', [128, 128], BF16)
    identf = dinp('identf', [128, 128], F32)
    onesd = dinp('onesd', [128, 128], F32R)
    zeros16 = dinp('zeros16', [128, 16], BF16)
    vztmpl = dinp('vztmpl', [128, 4, 4, 128], BF16)
    fvec = dinp('fvec', [128, 4], F32)
    for l in range(L):
        dinp(f'wqkvo{l}', [128, 3, DB, D], BF16)
        dinp(f'woP{l}', [128, 2, 2, D], BF16)
        dinp(f'wff1{l}', [128, DB, FF], BF16)
        dinp(f'wff2{l}', [128, 8, D], BF16)
        dinp(f'bvb{l}', [128, D], BF16)
        dinp(f'pvec{l}', [128, 16], F32)
        dinp(f'estrip{l}', [128, NE, 1032], BF16)
        dinp(f'sstrip{l}', [128, NS, 1032], F32)
    out = nc.dram_tensor('out', [BLOC, S, D], F32, kind='ExternalOutput')

    tc_cm = tile.TileContext(nc)
    tc = tc_cm.__enter__()
    cst = tc.alloc_tile_pool(name='cst', bufs=1)
    wp = tc.alloc_tile_pool(name='wp', bufs=2)
    ap = tc.alloc_tile_pool(name='ap', bufs=1)
    tp = tc.alloc_tile_pool(name='tp', bufs=2)
    vzp = tc.alloc_tile_pool(name='vzp', bufs=2)
    prp = tc.alloc_tile_pool(name='prp', bufs=6)
    h1p = tc.alloc_tile_pool(name='h1p', bufs=10)
    xip = tc.alloc_tile_pool(name='xip', bufs=4)
    zp = tc.alloc_tile_pool(name='zp', bufs=2)
    cnp = tc.alloc_tile_pool(name='cnp', bufs=4)
    ps = tc.alloc_tile_pool(name='ps', bufs=2, space='PSUM')

    # ---- consts
    identb_s = cst.tile([128, 128], BF16)
    nc.sync.dma_start(identb_s[:], identb[:])
    identf_s = cst.tile([128, 128], F32)
    nc.sync.dma_start(identf_s[:], identf[:])
    onesd_s = cst.tile([128, 128], F32R)
    nc.sync.dma_start(onesd_s[:], onesd[:])
    z16_s = cst.tile([128, 16], BF16)
    nc.sync.dma_start(z16_s[:], zeros16[:])
    cvec_s = cst.tile([128, DB, 4], F32)
    nc.sync.dma_start(cvec_s[:], cvec[:])
    fvec_s = cst.tile([128, 4], F32)
    nc.sync.dma_start(fvec_s[:], fvec[:])
    peT_s = cst.tile([128, DB, S], F32)
    nc.sync.dma_start(peT_s[:], peT[:])
    eps_s = cst.tile([128, 1], F32)
    nc.vector.memset(eps_s[:], EPS)
    w1A_s = cst.tile([128, D], BF16)
    nc.sync.dma_start(w1A_s[:], w1A[:])
    w1B_s = cst.tile([96, D], BF16)
    nc.sync.dma_start(w1B_s[:], w1B[:])
    w2t_s = cst.tile([128, DB, 5, D], BF16)
    nc.sync.dma_start(w2t_s[:], w2t[:])

    # vz slot templates (zeros + Z-ones). v columns are rewritten per use;
    # the static template regions persist across pool-slot reuse.
    for i in range(2):
        vzt_t = vzp.tile([128, 4, 4, 128], BF16, tag='vz', name=f'vzi{i}')
        nc.sync.dma_start(vzt_t[:], vztmpl[:])
    # zrec slots: Z-reciprocal rows are stream-shuffled in per use; the junk
    # rows (32:64, 96:128) are memset once so the out-of-band lanes of the
    # normalize multiply stay finite (their woP rows are zero).
    for i in range(2):
        zr_t = zp.tile([128, 2, 512], F32, tag='zrec', name=f'zri{i}')
        nc.vector.memset(zr_t[32:64, :, :], 1.0)
        nc.vector.memset(zr_t[96:128, :, :], 1.0)

    # persistent residual stream, feature-major [d mod 128, d//128, token]
    rt = ap.tile([128, DB, TOK], F32R)

    def emit_body(R):
        # ---------------- conv tokenizer (conv1 fp32, conv2 bf16)
        xts = {}
        for b in range(BLOC):
            X4 = xip.tile([128, 512], BF16, tag='xi', name=f'{R}x4_{b}')
            nc.sync.dma_start(X4[0:32, 3:512], x[b, :, 0:509])
            nc.sync.dma_start(X4[32:64, 2:512], x[b, :, 0:510])
            nc.sync.dma_start(X4[64:96, 1:512], x[b, :, 0:511])
            nc.sync.dma_start(X4[96:128, 0:512], x[b, :, 0:512])
            nc.sync.dma_start(X4[0:32, 0:3], z16_s[0:32, 0:3])
            nc.sync.dma_start(X4[32:64, 0:2], z16_s[32:64, 0:2])
            nc.sync.dma_start(X4[64:96, 0:1], z16_s[64:96, 0:1])
            X3 = xip.tile([128, 512], BF16, tag='xi', name=f'{R}x3_{b}')
            nc.sync.dma_start(X3[0:32, 0:511], x[b, :, 1:512])
            nc.sync.dma_start(X3[32:64, 0:510], x[b, :, 2:512])
            nc.sync.dma_start(X3[64:96, 0:509], x[b, :, 3:512])
            nc.sync.dma_start(X3[0:32, 511:512], z16_s[0:32, 0:1])
            nc.sync.dma_start(X3[32:64, 510:512], z16_s[32:64, 0:2])
            nc.sync.dma_start(X3[64:96, 509:512], z16_s[64:96, 0:3])
            xts[b] = (X4, X3)
        for b in range(BLOC):
            X4, X3 = xts[b]
            hp = tp.tile([128, DB, 516], BF16, tag='hp', name=f'{R}hp_{b}')
            c1 = ps.tile([128, 2, 512], F32, tag='sc', name=f'{R}c1_{b}')
            for dc in range(DB):
                nc.tensor.matmul(c1[:, dc, :], w1A_s[:, dc * 128:(dc + 1) * 128],
                                 X4[:], start=True, stop=False,
                                 skip_group_check=True)
                nc.tensor.matmul(c1[:, dc, :], w1B_s[:, dc * 128:(dc + 1) * 128],
                                 X3[0:96, :], start=False, stop=True,
                                 skip_group_check=True)
                nc.gpsimd.memset(hp[:, dc, 0:2], 0.0)
                nc.gpsimd.memset(hp[:, dc, 514:516], 0.0)
                nc.scalar.activation(hp[:, dc, 2:514], c1[:, dc, :], AF.Gelu,
                                     bias=cvec_s[:, dc, 1:2],
                                     scale=cvec_s[:, dc, 0:1])
            c2 = ps.tile([128, 2, 512], F32, tag='sc', name=f'{R}c2_{b}')
            for dc in range(DB):
                for cb in range(DB):
                    for k in range(5):
                        nc.tensor.matmul(
                            c2[:, dc, :], w2t_s[:, cb, k, dc * 128:(dc + 1) * 128],
                            hp[:, cb, k:k + 512],
                            start=(cb == 0 and k == 0),
                            stop=(cb == 1 and k == 4), skip_group_check=True)
                tg = h1p.tile([128, 512], BF16, tag='h1', name=f'{R}tg_{b}_{dc}')
                nc.scalar.activation(tg[:], c2[:, dc, :], AF.Gelu,
                                     bias=cvec_s[:, dc, 3:4],
                                     scale=cvec_s[:, dc, 2:3])
                nc.vector.tensor_add(rt[:, dc, b * S:(b + 1) * S],
                                     tg[:], peT_s[:, dc, :])

        # ---------------- per-(layer, b) stage emitters
        def layernorm(b, xn_t, tag, fin_stats=None):
            sl = slice(b * S, (b + 1) * S)
            sq = tp.tile([128, DB, 512], F32R, tag='sq', name=f'{R}sq_{tag}', bufs=1)
            nc.gpsimd.tensor_tensor(sq[:], rt[:, :, sl], rt[:, :, sl], ALU.mult)
            st = ps.tile([128, 2, 512], F32, tag='sc', name=f'{R}st_{tag}')
            for db in range(DB):
                nc.tensor.matmul(st[:, 0, :], onesd_s[:], rt[:, db, sl],
                                 start=(db == 0), stop=(db == 1),
                                 skip_group_check=True)
            for db in range(DB):
                nc.tensor.matmul(st[:, 1, :], onesd_s[:], sq[:, db, :],
                                 start=(db == 0), stop=(db == 1),
                                 skip_group_check=True)
            m2 = tp.tile([128, 512], F32, tag='lns', name=f'{R}m2_{tag}')
            nc.scalar.activation(m2[:], st[:, 0, :], AF.Square)
            var = tp.tile([128, 512], F32, tag='lns', name=f'{R}var_{tag}')
            nc.vector.tensor_sub(var[:], st[:, 1, :], m2[:])
            lnv = tp.tile([128, 512], F32, tag='lnv', name=f'{R}lnv_{tag}', bufs=1)
            nc.scalar.activation(lnv[:], var[:], AF.Ln, bias=eps_s[:, 0:1])
            rstd = tp.tile([128, 512], F32, tag='rstd', name=f'{R}rs_{tag}')
            nc.scalar.activation(rstd[:], lnv[:], AF.Exp, scale=-0.5)
            nmdt = F32 if fin_stats is not None else BF16
            nm = tp.tile([128, 512], nmdt, tag='nm', name=f'{R}nm_{tag}')
            nc.vector.scalar_tensor_tensor(nm[:], st[:, 0, :], -1.0, rstd[:],
                                           ALU.mult, ALU.mult)
            if fin_stats is not None:
                fin_stats.append((rstd, nm))
                return
            for db in range(DB):
                t1 = tp.tile([128, 512], BF16, tag='t1', name=f'{R}t1_{tag}{db}')
                nc.gpsimd.tensor_tensor(t1[:], rt[:, db, sl], rstd[:], ALU.mult)
                nc.vector.tensor_add(xn_t[:, db, sl], t1[:], nm[:])

        def qk(l, b, xn, qT, kT, wqkvo_s, pvec_s):
            sl = slice(b * S, (b + 1) * S)
            for mat, (dst, bc) in enumerate([(qT, 0), (kT, 2)]):
                mp = ps.tile([128, 2, 512], F32, tag='sc',
                             name=f'{R}qk{l}{b}{mat}')
                for mb in range(DB):
                    for kb in range(DB):
                        nc.tensor.matmul(
                            mp[:, mb, :],
                            wqkvo_s[:, mat, kb, mb * 128:(mb + 1) * 128],
                            xn[:, kb, sl], start=(kb == 0), stop=(kb == 1),
                            skip_group_check=True)
                for mb in range(DB):
                    nc.scalar.activation(
                        dst[:, mb, sl], mp[:, mb, :], AF.Identity,
                        bias=pvec_s[:, bc + mb:bc + mb + 1])

        def vproj(l, b, xn, vzs, wqkvo_s, bvb_s):
            for jc in range(4):
                vp = ps.tile([128, 256], F32, tag='sc', name=f'{R}v{l}{b}{jc}')
                nc.tensor.matmul(vp[:], identb_s[:], bvb_s[:],
                                 start=True, stop=False, skip_group_check=True)
                for kb in range(DB):
                    nc.tensor.matmul(
                        vp[:],
                        xn[:, kb, b * S + jc * 128:b * S + (jc + 1) * 128],
                        wqkvo_s[:, 2, kb, :], start=False, stop=(kb == 1),
                        skip_group_check=True)
                vp_r = vp.rearrange('p (hb he pc) -> p hb he pc', hb=2, pc=64)
                for hb in range(2):
                    vz_r = vzs[hb].rearrange(
                        'p jc (he two) m -> p jc he two m', two=2)
                    for par in range(2):
                        nc.vector.tensor_copy(
                            vz_r[:, jc, :, par, 64 * par:64 * par + 32],
                            vp_r[:, hb, :, 32 * par:32 * par + 32])

        def attn_pair(l, b, qT, kT, vzs, estrip_s, sstrip_s):
            """Both hb groups of one batch element, jc-interleaved so the PE
            fills one group's probs latency with the other group's matmuls."""
            czs = [ps.tile([128, 2, 512], F32, tag='cz',
                           name=f'{R}cz{l}{b}{hb}') for hb in range(2)]
            scs = {}
            for jc in range(4):
                off = 512 - jc * 128
                for hb in range(2):
                    scA = ps.tile([128, 2, 512], F32, tag='sc',
                                  name=f'{R}scA{l}{b}{hb}{jc}')
                    scB = ps.tile([128, 2, 512], F32, tag='sc',
                                  name=f'{R}scB{l}{b}{hb}{jc}')
                    scs[hb] = (scA, scB)
                    for hh in range(4):
                        dstp = scA if hh < 2 else scB
                        nc.tensor.matmul(
                            dstp[:, hh % 2, :],
                            kT[32 * hh:32 * hh + 32, hb,
                               b * S + jc * 128:b * S + (jc + 1) * 128],
                            qT[32 * hh:32 * hh + 32, hb, b * S:(b + 1) * S],
                            start=True, stop=True,
                            tile_position=(32 * hh, 0), skip_group_check=True)
                for hb in range(2):
                    scA, scB = scs[hb]
                    prA = prp.tile([128, 2, 512], BF16, tag='pr',
                                   name=f'{R}pr{l}{b}{hb}{jc}', bufs=2)
                    nc.scalar.activation(prA[:], scA[:], AF.Exp)
                    prm = prp.tile([128, 2, 512], BF16, tag='prm',
                                   name=f'{R}pm{l}{b}{hb}{jc}', bufs=2)
                    eng = nc.gpsimd if jc in MULT_GPS_JC else nc.vector
                    eng.tensor_tensor(
                        prm[:], prA[:],
                        estrip_s[:, 2 * hb:2 * hb + 2, off:off + 512],
                        ALU.mult)
                    prB = prp.tile([128, 2, 512], I16, tag='pri',
                                   name=f'{R}pi{l}{b}{hb}{jc}', bufs=4)
                    nc.vector.scalar_tensor_tensor(
                        prB[:], scB[:], SCA,
                        sstrip_s[:, 2 * hb:2 * hb + 2, off:off + 512],
                        ALU.mult, ALU.add)
                    prBb = prB.bitcast(BF16)
                    probs = [prm[:, 0, :], prm[:, 1, :], prBb[:, 0, :],
                             prBb[:, 1, :]]
                    for hh in range(4):
                        nc.tensor.matmul(
                            czs[hb][:, hh // 2, :], vzs[hb][:, jc, hh, :],
                            probs[hh],
                            start=(jc == 0 and hh % 2 == 0),
                            stop=(jc == 3 and hh % 2 == 1),
                            skip_group_check=True)
            # normalize both groups
            ctxns = []
            ident32 = list(range(32))
            for hb in range(2):
                cz = czs[hb]
                rec = tp.tile([128, 2, 512], F32, tag='rec',
                              name=f'{R}rc{l}{b}{hb}')
                nc.vector.reciprocal_approx_fast(rec[:], cz[:])
                zrec = zp.tile([128, 2, 512], F32, tag='zrec',
                               name=f'{R}zr{l}{b}{hb}')
                nc.vector.stream_shuffle(zrec[0:32, :, :], rec[32:64, :, :],
                                         ident32)
                nc.vector.stream_shuffle(zrec[64:96, :, :], rec[96:128, :, :],
                                         ident32)
                ctxn = cnp.tile([128, 2, 512], BF16, tag='ctxn',
                                name=f'{R}cn{l}{b}{hb}')
                nc.vector.tensor_mul(ctxn[:], cz[:], zrec[:])
                ctxns.append(ctxn)
            return ctxns

        def outproj(l, b, ctxns, woP_s, pvec_s):
            sl = slice(b * S, (b + 1) * S)
            op = ps.tile([128, 2, 512], F32, tag='sc', name=f'{R}op{l}{b}')
            for mb in range(DB):
                for hb in range(2):
                    for pb in range(2):
                        nc.tensor.matmul(
                            op[:, mb, :],
                            woP_s[:, hb, pb, mb * 128:(mb + 1) * 128],
                            ctxns[hb][:, pb, :],
                            start=(hb == 0 and pb == 0),
                            stop=(hb == 1 and pb == 1),
                            skip_group_check=True)
            for mb in range(DB):
                nc.vector.scalar_tensor_tensor(
                    rt[:, mb, sl], op[:, mb, :], pvec_s[:, 4 + mb:5 + mb],
                    rt[:, mb, sl], ALU.add, ALU.add)

        def ffn(l, b, xn2, wff1_s, wff2_s, pvec_s):
            sl = slice(b * S, (b + 1) * S)
            h1s = []
            for fp in range(4):
                hp_ = ps.tile([128, 2, 512], F32, tag='sc',
                              name=f'{R}h1{l}{b}{fp}')
                for half in range(2):
                    fb = 2 * fp + half
                    for kb in range(DB):
                        nc.tensor.matmul(
                            hp_[:, half, :],
                            wff1_s[:, kb, fb * 128:(fb + 1) * 128],
                            xn2[:, kb, sl], start=(kb == 0), stop=(kb == 1),
                            skip_group_check=True)
                for half in range(2):
                    fb = 2 * fp + half
                    h1t = h1p.tile([128, 512], BF16, tag='h1',
                                   name=f'{R}h1t{l}{b}{fb}')
                    nc.scalar.activation(h1t[:], hp_[:, half, :], AF.Gelu,
                                         bias=pvec_s[:, 8 + fb:9 + fb])
                    h1s.append(h1t)
            f2 = ps.tile([128, 2, 512], F32, tag='sc', name=f'{R}f2{l}{b}')
            for db in range(DB):
                for fb in range(8):
                    nc.tensor.matmul(
                        f2[:, db, :], wff2_s[:, fb, db * 128:(db + 1) * 128],
                        h1s[fb][:], start=(fb == 0), stop=(fb == 7),
                        skip_group_check=True)
            for db in range(DB):
                nc.vector.scalar_tensor_tensor(
                    rt[:, db, sl], f2[:, db, :], pvec_s[:, 6 + db:7 + db],
                    rt[:, db, sl], ALU.add, ALU.add)

        # ---------------- transformer layers
        if upto == 'conv':
            sink = h1p.tile([128, 256], F32, tag='h1', name=f'{R}sink')
            nc.vector.tensor_copy(sink[:], rt[:, 0, 0:256])
            nc.sync.dma_start(out[0, 0:128, :], sink[:])
            return
        for l in range(L):
            wqkvo_s = wp.tile([128, 3, DB, D], BF16, tag='wqkvo', name=f'{R}wm{l}')
            nc.sync.dma_start(wqkvo_s[:], din[f'wqkvo{l}'][:])
            woP_s = wp.tile([128, 2, 2, D], BF16, tag='woP', name=f'{R}wo{l}')
            nc.sync.dma_start(woP_s[:], din[f'woP{l}'][:])
            wff1_s = wp.tile([128, DB, FF], BF16, tag='wff1', name=f'{R}w1{l}')
            nc.sync.dma_start(wff1_s[:], din[f'wff1{l}'][:])
            wff2_s = wp.tile([128, 8, D], BF16, tag='wff2', name=f'{R}w2{l}')
            nc.sync.dma_start(wff2_s[:], din[f'wff2{l}'][:])
            bvb_s = wp.tile([128, D], BF16, tag='bvb', name=f'{R}bv{l}')
            nc.sync.dma_start(bvb_s[:], din[f'bvb{l}'][:])
            pvec_s = wp.tile([128, 16], F32, tag='pvec', name=f'{R}pv{l}')
            nc.sync.dma_start(pvec_s[:], din[f'pvec{l}'][:])
            estrip_s = wp.tile([128, NE, 1032], BF16, tag='estrip',
                               name=f'{R}es{l}', bufs=1)
            nc.sync.dma_start(estrip_s[:], din[f'estrip{l}'][:])
            sstrip_s = wp.tile([128, NS, 1032], F32, tag='sstrip',
                               name=f'{R}ss{l}', bufs=1)
            nc.sync.dma_start(sstrip_s[:], din[f'sstrip{l}'][:])

            xn = tp.tile([128, DB, TOK], BF16, tag='xn', name=f'{R}xn{l}')
            qT = ap.tile([128, DB, TOK], BF16, tag='qT', name=f'{R}qT{l}')
            kT = ap.tile([128, DB, TOK], BF16, tag='kT', name=f'{R}kT{l}')
            for b in range(BLOC):
                layernorm(b, xn, f'a{l}{b}')
            for b in range(BLOC):
                qk(l, b, xn, qT, kT, wqkvo_s, pvec_s)
            vz0 = [vzp.tile([128, 4, 4, 128], BF16, tag='vz',
                            name=f'{R}vz{l}0{hb}') for hb in range(2)]
            vproj(l, 0, xn, vz0, wqkvo_s, bvb_s)
            cns0 = attn_pair(l, 0, qT, kT, vz0, estrip_s, sstrip_s)
            vz1 = [vzp.tile([128, 4, 4, 128], BF16, tag='vz',
                            name=f'{R}vz{l}1{hb}') for hb in range(2)]
            vproj(l, 1, xn, vz1, wqkvo_s, bvb_s)
            cns1 = attn_pair(l, 1, qT, kT, vz1, estrip_s, sstrip_s)
            outproj(l, 0, cns0, woP_s, pvec_s)
            xn2 = tp.tile([128, DB, TOK], BF16, tag='xn', name=f'{R}xn2_{l}')
            layernorm(0, xn2, f'f{l}0')
            outproj(l, 1, cns1, woP_s, pvec_s)
            layernorm(1, xn2, f'f{l}1')
            ffn(l, 0, xn2, wff1_s, wff2_s, pvec_s)
            ffn(l, 1, xn2, wff1_s, wff2_s, pvec_s)

        if upto != 'full':
            sink = h1p.tile([128, 256], F32, tag='h1', name=f'{R}sink')
            nc.vector.tensor_copy(sink[:], rt[:, 0, 0:256])
            nc.sync.dma_start(out[0, 0:128, :], sink[:])
            return
        # ---------------- final LN (+affine) and transpose to token-major
        fin = tp.tile([128, DB, TOK], F32, tag='fin', name=f'{R}fin', bufs=1)
        stats = []
        for b in range(BLOC):
            layernorm(b, None, f'fin{b}', fin_stats=stats)
        for b in range(BLOC):
            sl = slice(b * S, (b + 1) * S)
            rstd, nm = stats[b]
            for db in range(DB):
                rstd_g = tp.tile([128, 512], F32, tag='rstd_g',
                                 name=f'{R}rg{b}{db}', bufs=1)
                nc.vector.tensor_scalar(rstd_g[:], rstd[:],
                                        fvec_s[:, 0 + db:1 + db], None, ALU.mult)
                nm_gb = tp.tile([128, 512], F32, tag='nm_gb', name=f'{R}ng{b}{db}', bufs=1)
                nc.vector.tensor_scalar(nm_gb[:], nm[:],
                                        fvec_s[:, 0 + db:1 + db],
                                        fvec_s[:, 2 + db:3 + db],
                                        ALU.mult, ALU.add)
                t1 = tp.tile([128, 512], F32, tag='ft1', name=f'{R}ft1{b}{db}', bufs=1)
                nc.gpsimd.tensor_tensor(t1[:], rt[:, db, sl], rstd_g[:],
                                        ALU.mult)
                nc.vector.tensor_add(fin[:, db, sl], t1[:], nm_gb[:])
        for b in range(BLOC):
            for jc in range(4):
                tc_sl = slice(b * S + jc * 128, b * S + (jc + 1) * 128)
                pst = ps.tile([128, 256], F32, tag='sc', name=f'{R}tr{b}{jc}')
                for db in range(DB):
                    nc.tensor.transpose(pst[:, db * 128:(db + 1) * 128],
                                        fin[:, db, tc_sl], identf_s[:])
                osb = h1p.tile([128, 256], F32, tag='h1', name=f'{R}ot{b}{jc}')
                nc.vector.tensor_copy(osb[:], pst[:])
                nc.sync.dma_start(out[b, jc * 128:(jc + 1) * 128, :], osb[:])

    for _rep in range(repeat):
        emit_body(f'r{_rep}_')

    for pool in [ps, cnp, zp, xip, h1p, prp, vzp, tp, ap, wp, cst]:
        pool.release()
    tc_cm.__exit__(None, None, None)
    nc.finalize()
    return nc


# ---------------------------------------------------------------- entry point
def kernel(**inputs):
    p = _prep(inputs)
    if 'nc' not in _CACHE:
        _CACHE['nc'] = _build()
    nc = _CACHE['nc']
    x = np.ascontiguousarray(
        np.asarray(inputs['x'], np.float32).astype(ml_dtypes.bfloat16))
    in_maps = []
    for c in range(NCORES):
        m = dict(p)
        m['x'] = np.ascontiguousarray(x[c * BLOC:(c + 1) * BLOC])
        in_maps.append(m)
    res = run_bass_kernel_spmd(nc, in_maps, core_ids=list(range(NCORES)),
                               trace=TRACE)
    out = np.concatenate([r['out'] for r in res.results], axis=0)
    kernel.last_results = res
    return np.ascontiguousarray(out.astype(np.float32))
